# revision 2
# baseline (speedup 1.0000x reference)
"""Trainium2 Bass kernel for nn_LlamaAttentionPNA_LM (v3 redesign).

Sharding: 8 cores, 2 heads per core (tensor-parallel over heads). Each core
computes its 2 heads end-to-end plus a partial o_proj over the full output;
the host sums the 8 partials.

Selection (per head, per 128-row chunk c, candidate width W=128(c+1)):
  scores (PE fp32) -> row moments (ACT accum on PSUM) -> Gaussian cutoff
  t_est -> mask + prefix-scan + local_scatter compaction to CAP~2.5k ->
  max8/match_replace rounds on the narrow tile -> one-hot dot extracts the
  k-th largest T -> adj = (g >= T) directly as bf16.
  Chunks 0-1 run rounds directly on the threshold-filtered values (exact
  reference semantics incl. below-threshold index-ordered fill); chunks 2+
  run on raw scores (validated: enough above-threshold candidates).

Aggregation: sum/sumsq via bf16 matmuls (adjT x [v, v^2]); max aggregator
via per-j-block log-sum-exp matmuls: E = exp(beta(v - M_block)) (bf16),
sB = adjT_block @ E, mx = max_b(log(sB)/beta + M_block). Chunk 0 uses an
exact ap_gather path (tiny k). GIN MLP and o_proj in bf16.
"""

import numpy as np
from contextlib import ExitStack

import concourse.bass as bass
from concourse import bacc
import concourse.mybir as mybir
import concourse.tile as tile
from concourse.masks import make_identity

F32 = mybir.dt.float32
BF16 = mybir.dt.bfloat16
U8 = mybir.dt.uint8
I16 = mybir.dt.int16
U16 = mybir.dt.uint16

H, D, HID, S = 16, 64, 1024, 1024
MULT = 2
FRAC, THR, BASE = 0.1, 0.2, 10000.0
NEG = -1e30
DELTA = 1e-8
NCHUNK = S // 128
NCORES = 8
DEBUG = False
BETA = 24.0
LNS = 32.0     # Ln input prescale (ACT Ln is accurate only in [e^-40, e^40])
ECLIP = 70.0   # Exp-input clamp so sB spans <= ~75 e-folds
MXGUARD = -30.0

AX = mybir.AxisListType.X
ALU = mybir.AluOpType
AF = mybir.ActivationFunctionType


def _k_vec():
    k = np.ceil(np.float32(FRAC) * np.arange(S, dtype=np.float32)).astype(np.int64)
    k = np.maximum(k, 1)
    k[0] = 0
    return k


KV = _k_vec()
KMAXC = [int(KV[128 * (c + 1) - 1]) for c in range(NCHUNK)]
RC = [(km + 7) // 8 for km in KMAXC]          # max8 rounds per chunk
CAPS = [0, 0, 80, 104, 128, 160, 184, 208]    # est-compaction caps (c>=2)
OHW = 112                                     # one-hot table width (>= 8*R)
KP0 = 16                                      # chunk-0 gather pad
# blob column layout (f32 units): zr 256 | rden 1024 | ohm 896 | zqt 8 |
# eps 1 | pmat 128 | iot(i16 x128 ->) 64
BO_ZR, BO_RD, BO_OHM, BO_ZQ, BO_EPS, BO_PM, BO_IOT = 0, 256, 1280, 2176, 2184, 2185, 2313
BLOBW = 2377


def _build_nc():
    nc = bacc.Bacc("TRN2", target_bir_lowering=False, debug=False,
                   num_devices=NCORES)

    din = {}

    def inp(name, shape, dt=F32):
        din[name] = nc.dram_tensor(name, list(shape), dt, kind="ExternalInput").ap()
        return din[name]

    hsT = inp("hsT", (HID, S))
    wq = inp("wq", (HID, 128))
    wk = inp("wk", (HID, 128))
    wv = inp("wv", (HID, 128))
    wob = inp("wob", (128, S), BF16)
    w1b = inp("w1b", (2, 4 * D, MULT * D), BF16)
    w2b = inp("w2b", (2, MULT * D, D), BF16)
    ropes = inp("ropes", (128, 4 * S))        # [tck|tsk|tcq|tsq]
    blob = inp("blob", (128, BLOBW))          # packed small tables

    outp = nc.dram_tensor("outp", [S, S], BF16, kind="ExternalOutput").ap()
    dbg = {}
    if DEBUG:
        for nm in ("d_sum", "d_mean", "d_mx", "d_var", "d_hout"):
            dbg[nm] = nc.dram_tensor(nm, [128, S], F32,
                                     kind="ExternalOutput").ap()
        dbg["d_t"] = nc.dram_tensor("d_t", [128, NCHUNK * 2], F32,
                                    kind="ExternalOutput").ap()
        dbg["d_test"] = nc.dram_tensor("d_test", [128, NCHUNK * 2], F32,
                                       kind="ExternalOutput").ap()

    with tile.TileContext(nc) as tc, ExitStack() as ctx:
        # ---------------- persistent tiles ----------------
        pers = ctx.enter_context(tc.tile_pool(name="pers", bufs=1))
        qTr = pers.tile([128, S], F32, tag="qTr")
        kTr = pers.tile([128, S], F32, tag="kTr")
        vT = pers.tile([128, S], F32, tag="vT")
        Ebf = pers.tile([128, S], BF16, tag="Ebf")
        mbneg = pers.tile([128, NCHUNK], F32, tag="mbneg")   # -beta*Mb
        mbpos = pers.tile([128, NCHUNK], F32, tag="mbpos")   # Mb - LNS/beta
        epsv = pers.tile([128, S], F32, tag="epsv")
        comb_sum = pers.tile([128, S], BF16, tag="comb_sum")
        comb_mean = pers.tile([128, S], BF16, tag="comb_mean")
        comb_mx = pers.tile([128, S], BF16, tag="comb_mx")
        comb_var = pers.tile([128, S], BF16, tag="comb_var")
        houtT = pers.tile([128, S], BF16, tag="houtT")
        identb = pers.tile([128, 128], BF16, tag="identb")
        identf = pers.tile([128, 128], F32, tag="identf")
        vTg0 = pers.tile([128, 1 + 128], F32, tag="vTg0")
        v_all = [pers.tile([128, 256], BF16, tag=f"v_all{jb}", name=f"v_all{jb}")
                 for jb in range(NCHUNK)]
        e_all = [pers.tile([128, 128], BF16, tag=f"e_all{jb}", name=f"e_all{jb}")
                 for jb in range(NCHUNK)]
        adjT = [[pers.tile([128, S - 128 * jb], BF16, tag=f"adjT{h}_{jb}",
                           name=f"adjT{h}_{jb}")
                 for jb in range(NCHUNK)] for h in range(2)]

        make_identity(nc, identb[:])
        make_identity(nc, identf[:])
        blobt = pers.tile([128, BLOBW], F32, tag="blobt")


        # ---- DMA spread across engine queues ----
        _qs = [nc.sync]
        _qi = [0]

        def dma(dst, src):
            eng = _qs[_qi[0] % len(_qs)]
            _qi[0] += 1
            eng.dma_start(dst, src)

        # ---------------- phase A prologue: weights + hs + tables ----------
        aw = ctx.enter_context(tc.tile_pool(name="aw", bufs=1))
        hspool = ctx.enter_context(tc.tile_pool(name="hs", bufs=1))
        rtab = ctx.enter_context(tc.tile_pool(name="ropetab", bufs=1))

        ropet = rtab.tile([128, 4 * S], F32, tag="ropet")
        tk = ropet[:, 0:S]
        tsk_t = ropet[:, S:2 * S]
        tq = ropet[:, 2 * S:3 * S]
        tsq_t = ropet[:, 3 * S:4 * S]

        wqall = aw.tile([128, 8 * 128], F32, tag="wqall")
        wkall = aw.tile([128, 8 * 128], F32, tag="wkall")
        wvall = aw.tile([128, 8 * 128], F32, tag="wvall")
        hstall = hspool.tile([128, 8 * S], F32, tag="hstall")
        wqt = [wqall[:, 128 * k:128 * (k + 1)] for k in range(8)]
        wkt = [wkall[:, 128 * k:128 * (k + 1)] for k in range(8)]
        wvt = [wvall[:, 128 * k:128 * (k + 1)] for k in range(8)]
        hst = [hstall[:, S * k:S * (k + 1)] for k in range(8)]
        for k in range(8):
            dma(wkall[:, 128 * k:128 * (k + 1)], wk[128 * k:128 * (k + 1), :])
            dma(wqall[:, 128 * k:128 * (k + 1)], wq[128 * k:128 * (k + 1), :])
        for k in range(8):
            dma(hstall[:, S * k:S * k + 256], hsT[128 * k:128 * (k + 1), 0:256])
        dma(ropet[:, 0:S], ropes[:, 0:S])
        dma(ropet[:, 2 * S:3 * S], ropes[:, 2 * S:3 * S])
        dma(blobt[:], blob)
        dma(ropet[:, S:2 * S], ropes[:, S:2 * S])
        dma(ropet[:, 3 * S:4 * S], ropes[:, 3 * S:4 * S])
        for k in range(8):
            dma(hstall[:, S * k + 256:S * (k + 1)],
                hsT[128 * k:128 * (k + 1), 256:S])
        for k in range(8):
            dma(wvall[:, 128 * k:128 * (k + 1)], wv[128 * k:128 * (k + 1), :])
        zr = blobt[:, BO_ZR:BO_ZR + 256]
        rd = blobt[:, BO_RD:BO_RD + 1024]
        ohmt = blobt[:, BO_OHM:BO_OHM + 896]
        zqtt = blobt[:, BO_ZQ:BO_ZQ + 8]
        epst = blobt[:, BO_EPS:BO_EPS + 1]
        pmtt = aw.tile([128, 128], F32, tag="pmtt")
        nc.vector.tensor_copy(pmtt[:], blobt[:, BO_PM:BO_PM + 128])
        pmt = pmtt[:]
        iott = aw.tile([128, 128], I16, tag="iott")
        nc.vector.tensor_copy(iott[:], blobt[:, BO_IOT:BO_IOT + 64].bitcast(I16))
        iot = iott[:]

        # ---------------- merged per-chunk pipeline ----------------
        scpsum = ctx.enter_context(tc.tile_pool(name="scps", bufs=2, space="PSUM"))
        mpsum = ctx.enter_context(tc.tile_pool(name="mps", bufs=4, space="PSUM"))
        gpool = ctx.enter_context(tc.tile_pool(name="gp", bufs=2))
        tkpool = ctx.enter_context(tc.tile_pool(name="tkp", bufs=2))
        smallp = ctx.enter_context(tc.tile_pool(name="smallp", bufs=4))
        dscr = ctx.enter_context(tc.tile_pool(name="dscr", bufs=2, space="DRAM"))
        gatp = ctx.enter_context(tc.tile_pool(name="gatp", bufs=2))
        ropep = ctx.enter_context(tc.tile_pool(name="ropep", bufs=2))

        def proj_rope(c, wt, dstT, ctab, stab):
            """project chunk c of q/k and apply rope into dstT[:, cc]."""
            cc = slice(128 * c, 128 * (c + 1))
            pp = mpsum.tile([128, 128], F32, tag="ps1")
            for k in range(8):
                nc.tensor.matmul(pp[:], lhsT=wt[k][:], rhs=hst[k][:, cc],
                                 start=(k == 0), stop=(k == 7))
            xsb = ropep.tile([128, 128], F32, tag="ropex")
            nc.scalar.copy(xsb[:], pp[:])
            rps = mpsum.tile([128, 128], F32, tag="ps1")
            nc.tensor.matmul(rps[:], lhsT=pmt, rhs=xsb[:], start=True,
                             stop=True)
            rot = ropep.tile([128, 128], F32, tag="roper")
            nc.scalar.copy(rot[:], rps[:])
            nc.vector.tensor_tensor(dstT[:, cc], xsb[:], ctab[:, cc],
                                    op=ALU.mult)
            nc.vector.tensor_tensor(rot[:], rot[:], stab[:, cc], op=ALU.mult)
            nc.vector.tensor_tensor(dstT[:, cc], dstT[:, cc], rot[:],
                                    op=ALU.add)

        tvals = {}
        adjsb = {}

        def sel_chunk(c):
            """scores + selection + adjT transposes for both heads of chunk c."""
            W = 128 * (c + 1)
            R = RC[c]
            CAP = CAPS[c]
            for h in range(2):
                po = 64 * h
                sc = scpsum.tile([128, W], F32, tag="sc")
                for n0 in range(0, W, 512):
                    n1 = min(n0 + 512, W)
                    nc.tensor.matmul(
                        sc[:, n0:n1],
                        lhsT=qTr[po:po + 64, 128 * c:128 * (c + 1)],
                        rhs=kTr[po:po + 64, n0:n1], start=True, stop=True)

                g = gpool.tile([128, W], F32, tag="g")
                if c <= 1:
                    # exact reference semantics: below-thr -> delta*(S-j)
                    scsb = gpool.tile([128, W], F32, tag="scsb")
                    nc.scalar.copy(scsb[:], sc[:])
                    msk = smallp.tile([128, W], U8, tag="msk")
                    nc.vector.tensor_scalar(msk[:], scsb[:], float(THR), None,
                                            op0=ALU.is_ge)
                    nc.scalar.copy(g[:], zr[:, 0:W])
                    nc.vector.copy_predicated(g[:], msk[:], scsb[:])
                else:
                    # moments over the full [128, W] psum scores (in-place
                    # outs; the Square destroys sc after g is copied out)
                    s1 = smallp.tile([128, 1], F32, tag="s1")
                    s2 = smallp.tile([128, 1], F32, tag="s2")
                    nc.scalar.activation(sc[:], sc[:], AF.Copy, accum_out=s1[:])
                    nc.scalar.copy(g[:], sc[:])
                    nc.scalar.activation(sc[:], sc[:], AF.Square,
                                         accum_out=s2[:])
                    # t_est = max(mu + sd*z, 0.01)   (Pool engine, tiny ops)
                    mu = smallp.tile([128, 1], F32, tag="mu")
                    nc.vector.tensor_scalar(mu[:], s1[:], 1.0 / W, None,
                                            op0=ALU.mult)
                    mu2 = smallp.tile([128, 1], F32, tag="mu2")
                    nc.vector.tensor_tensor(mu2[:], mu[:], mu[:], op=ALU.mult)
                    var = smallp.tile([128, 1], F32, tag="varr")
                    nc.vector.tensor_scalar(var[:], s2[:], 1.0 / W, mu2[:, 0:1],
                                            op0=ALU.mult, op1=ALU.subtract)
                    sd = smallp.tile([128, 1], F32, tag="sd")
                    nc.scalar.activation(sd[:], var[:], AF.Sqrt)
                    tst = smallp.tile([128, 1], F32, tag="tst")
                    nc.vector.tensor_tensor(tst[:], sd[:], zqtt[:, c:c + 1],
                                            op=ALU.mult)
                    nc.vector.tensor_tensor(tst[:], tst[:], mu[:], op=ALU.add)
                    nc.vector.tensor_scalar(tst[:], tst[:], 0.01, None,
                                            op0=ALU.max)

                # causal NEG fill on the diagonal block
                nc.gpsimd.affine_select(
                    out=g[:, 128 * c:W], in_=g[:, 128 * c:W],
                    compare_op=ALU.is_gt, fill=float(NEG),
                    base=0, pattern=[[-1, 128]], channel_multiplier=1)

                if c >= 2:
                    # est-compaction: mask, prefix count, clamped scatter slots
                    m = gpool.tile([128, W], F32, tag="m")
                    nc.vector.tensor_scalar(m[:], g[:], tst[:, 0:1], None,
                                            op0=ALU.is_ge)
                    cnt = gpool.tile([128, W], F32, tag="cnt")
                    nc.vector.tensor_tensor_scan(
                        cnt[:], m[:], m[:], 0.0,
                        op0=ALU.add, op1=ALU.bypass)
                    t1 = gpool.tile([128, W], F32, tag="t1")
                    nc.vector.scalar_tensor_tensor(
                        t1[:], cnt[:], float(CAP), m[:], op0=ALU.is_le,
                        op1=ALU.mult)
                    scat = m
                    nc.vector.scalar_tensor_tensor(
                        scat[:], cnt[:], 1.0, t1[:], op0=ALU.mult, op1=ALU.mult)
                    # pair indices (2s, 2s+1) for 2-byte scatter of f32 g
                    sidx = tkpool.tile([128, 2 * W], I16, tag="sidx")
                    sv = sidx[:].rearrange("p (w two) -> p w two", two=2)
                    nc.vector.tensor_scalar(sv[:, :, 0:1], scat[:], 2.0, -2.0,
                                            op0=ALU.mult, op1=ALU.add)
                    nc.vector.tensor_scalar(sv[:, :, 1:2], scat[:], 2.0, -1.0,
                                            op0=ALU.mult, op1=ALU.add)
                    gc = tkpool.tile([128, 2 * max(CAP, 8 * R)], I16, tag="gc")
                    nc.gpsimd.local_scatter(
                        gc[:, 0:2 * CAP], g[:].bitcast(I16), sidx[:],
                        channels=128, num_elems=2 * CAP, num_idxs=2 * W)
                    gw = gc[:].bitcast(F32)
                    RW = CAP
                else:
                    gwt = tkpool.tile([128, max(W, 8 * R)], F32, tag="gwt")
                    nc.vector.tensor_copy(gwt[:, 0:W], g[:])
                    gw = gwt[:]
                    RW = W

                # max8/match_replace rounds to depth 8R
                vals = tkpool.tile([128, 8 * R], F32, tag="vals")
                for r in range(R):
                    sl = slice(8 * r, 8 * r + 8)
                    nc.vector.max(vals[:, sl], gw[:, 0:RW])
                    nc.vector.match_replace(gw[:, 0:RW], vals[:, sl],
                                            gw[:, 0:RW], float(NEG))

                # T = vals[k_i - 1] via fused one-hot dot
                tv = smallp.tile([128, OHW], F32, tag="tv")
                tthr = smallp.tile([128, 1], F32, tag="tthr")
                nc.vector.tensor_tensor(
                    tv[:, 0:8 * R], vals[:],
                    ohmt[:, c * OHW:c * OHW + 8 * R], op=ALU.mult)
                nc.vector.tensor_reduce(tthr[:], tv[:, 0:8 * R], axis=AX,
                                        op=ALU.add)
                if DEBUG:
                    nc.sync.dma_start(dbg["d_t"][0:128, 2 * c + h:2 * c + h + 1],
                                      tthr[:])
                    if c >= 2:
                        nc.sync.dma_start(
                            dbg["d_test"][0:128, 2 * c + h:2 * c + h + 1],
                            tst[:])

                # adjacency, bf16 (transposed next iteration)
                adj = gpool.tile([128, W], BF16, tag="adj", bufs=4)
                nc.vector.tensor_scalar(adj[:], g[:], tthr[:, 0:1], None,
                                        op0=ALU.is_ge)
                adjsb[(c, h)] = adj

                # chunk-0: index lists for the exact gather path (k <= 13)
                if c == 0:
                    cnt0 = smallp.tile([128, 128], F32, tag="cnt0")
                    nc.vector.tensor_tensor_scan(
                        cnt0[:], adj[:], adj[:], 0.0,
                        op0=ALU.add, op1=ALU.bypass)
                    t10 = smallp.tile([128, 128], F32, tag="t10")
                    nc.vector.scalar_tensor_tensor(
                        t10[:], cnt0[:], float(KP0), adj[:], op0=ALU.is_le,
                        op1=ALU.mult)
                    scat0 = smallp.tile([128, 128], F32, tag="scat0")
                    nc.vector.scalar_tensor_tensor(
                        scat0[:], cnt0[:], 1.0, t10[:], op0=ALU.mult,
                        op1=ALU.mult)
                    s0i = smallp.tile([128, 128], I16, tag="s0i")
                    nc.vector.tensor_scalar(s0i[:], scat0[:], 1.0, -1.0,
                                            op0=ALU.mult, op1=ALU.add)
                    ilist = smallp.tile([128, KP0], I16, tag="ilist")
                    nc.gpsimd.local_scatter(ilist[:], iot, s0i[:],
                                            channels=128, num_elems=KP0,
                                            num_idxs=128)
                    sc_dram = dscr.tile([128, KP0], I16, tag=f"scr{h}")
                    nc.sync.dma_start(sc_dram[0:128, 0:KP0], ilist[:])
                    tvals[(h, "ilist")] = sc_dram

        def vblock():
            """v projection and derived tables (vT, E, v_all, e_all)."""
            for n in range(2):
                sl = slice(512 * n, 512 * (n + 1))
                vp = mpsum.tile([128, 512], F32, tag="ps1")
                for k in range(8):
                    nc.tensor.matmul(vp[:], lhsT=wvt[k][:], rhs=hst[k][:, sl],
                                     start=(k == 0), stop=(k == 7))
                nc.scalar.copy(vT[:, sl], vp[:])
            nc.vector.tensor_scalar(epsv[:], vT[:], epst[:, 0:1], None,
                                    op0=ALU.mult)
            nc.vector.memset(vTg0[:, 0:1], NEG)
            nc.scalar.copy(vTg0[:, 1:129], vT[:, 0:128])
            for jb in range(NCHUNK):
                nc.vector.tensor_reduce(mbpos[:, jb:jb + 1],
                                        vT[:, 128 * jb:128 * (jb + 1)],
                                        axis=AX, op=ALU.max)
            nc.vector.tensor_scalar(mbneg[:], mbpos[:], -BETA, None,
                                    op0=ALU.mult)
            mbc = pers.tile([128, NCHUNK], F32, tag="mbc")
            nc.vector.tensor_scalar(mbc[:], mbpos[:], float(ECLIP / BETA),
                                    None, op0=ALU.subtract)
            nc.vector.tensor_scalar(mbpos[:], mbpos[:], float(LNS / BETA),
                                    None, op0=ALU.subtract)
            for jb in range(NCHUNK):
                vcl = ropep.tile([128, 128], F32, tag="vcl")
                nc.vector.tensor_scalar(vcl[:], vT[:, 128 * jb:128 * (jb + 1)],
                                        mbc[:, jb:jb + 1], None, op0=ALU.max)
                nc.scalar.activation(Ebf[:, 128 * jb:128 * (jb + 1)], vcl[:],
                                     AF.Exp, bias=mbneg[:, jb:jb + 1],
                                     scale=BETA)
            for jb in range(NCHUNK):
                tp2 = mpsum.tile([128, 128], F32, tag="ps1")
                nc.tensor.transpose(tp2[:], vT[:, 128 * jb:128 * (jb + 1)],
                                    identf[:])
                for h in range(2):
                    nc.scalar.copy(v_all[jb][:, 128 * h:128 * h + 64],
                                   tp2[:, 64 * h:64 * h + 64])
                    nc.scalar.activation(
                        v_all[jb][:, 128 * h + 64:128 * h + 128],
                        tp2[:, 64 * h:64 * h + 64], AF.Square)
                tpe = mpsum.tile([128, 128], BF16, tag="ps1")
                nc.tensor.transpose(tpe[:], Ebf[:, 128 * jb:128 * (jb + 1)],
                                    identb[:])
                nc.vector.tensor_copy(e_all[jb][:], tpe[:])

        def gather0():
            """chunk-0 exact max via ap_gather of the top-k v columns."""
            irep = gatp.tile([128, 8 * KP0], I16, tag="irep")
            for h in range(2):
                srcl = tvals[(h, "ilist")][0:128, 0:KP0]
                srcl = srcl.rearrange("(b q) s -> q b s", q=16)
                for gq in range(4):
                    g0 = (4 * h + gq) * 16
                    nc.sync.dma_start(
                        irep[g0:g0 + 16, :].rearrange("q (b s) -> q b s", b=8),
                        srcl)
            for b in range(8):
                gat = gatp.tile([128, 16 * KP0], F32, tag="gat")
                nc.gpsimd.ap_gather(
                    gat[:], vTg0[:], irep[:, b * KP0:(b + 1) * KP0],
                    channels=128, num_elems=129, d=1, num_idxs=16 * KP0)
                nc.vector.tensor_reduce(
                    comb_mx[:, 16 * b:16 * b + 16],
                    gat[:].rearrange("p (s r) -> p r s", r=16),
                    axis=AX, op=ALU.max)
            nc.vector.memset(comb_mx[:, 0:1], 0.0)

        def transp(c):
            for h in range(2):
                adj = adjsb.pop((c, h))
                for jb in range(c + 1):
                    tp = mpsum.tile([128, 128], BF16, tag="ps1")
                    nc.tensor.transpose(tp[:], adj[:, 128 * jb:128 * (jb + 1)],
                                        identb[:])
                    nc.vector.tensor_copy(
                        adjT[h][jb][:, 128 * (c - jb):128 * (c - jb) + 128],
                        tp[:])

        def phasec_mm(c):
            """aggregation matmuls + moments + LSE max for chunk c."""
            cc = slice(128 * c, 128 * (c + 1))
            for h in range(2):
                po = 64 * h
                pa = mpsum.tile([128, 128], F32, tag="ps1")
                for jb in range(c + 1):
                    lhs = v_all[jb][:, 128 * h:128 * (h + 1)]
                    nc.tensor.matmul(
                        pa[:], lhsT=lhs,
                        rhs=adjT[h][jb][:, 128 * (c - jb):128 * (c - jb) + 128],
                        start=(jb == 0), stop=(jb == c))
                nc.scalar.copy(comb_sum[po:po + 64, cc], pa[0:64, :])
                nc.vector.tensor_tensor(comb_mean[po:po + 64, cc], pa[0:64, :],
                                        rd[po:po + 64, cc], op=ALU.mult)
                varm = tmpp.tile([128, 128], F32, tag="varm")
                nc.vector.tensor_tensor(varm[po:po + 64, :], pa[64:128, :],
                                        rd[po:po + 64, cc], op=ALU.mult)
                msq = tmpp.tile([128, 128], F32, tag="msq")
                nc.scalar.activation(msq[po:po + 64, :],
                                     comb_mean[po:po + 64, cc], AF.Square)
                nc.vector.tensor_tensor(varm[po:po + 64, :],
                                        varm[po:po + 64, :],
                                        msq[po:po + 64, :], op=ALU.subtract)
                nc.vector.tensor_scalar(comb_var[po:po + 64, cc],
                                        varm[po:po + 64, :], 0.0, None,
                                        op0=ALU.max)

            # LSE max aggregator (chunks >= 1); sB oriented [d-part, i-free]
            if c >= 1:
                mxa = tmpp.tile([128, 128], BF16, tag="mxa")
                for jb in range(c + 1):
                    sB = mpsum.tile([128, 128], F32, tag="ps1")
                    for h in range(2):
                        nc.tensor.matmul(
                            sB[64 * h:64 * h + 64, :],
                            lhsT=e_all[jb][:, 64 * h:64 * h + 64],
                            rhs=adjT[h][jb][:,
                                            128 * (c - jb):128 * (c - jb) + 128],
                            start=True, stop=True)
                    lg = tmpp.tile([128, 128], F32, tag="lg")
                    nc.scalar.activation(lg[:], sB[:], AF.Ln,
                                         scale=float(np.exp(LNS)))
                    if jb == 0:
                        nc.vector.tensor_scalar(
                            mxa[:], lg[:], 1.0 / BETA, mbpos[:, 0:1],
                            op0=ALU.mult, op1=ALU.add)
                    else:
                        mxb = tmpp.tile([128, 128], BF16, tag="mxb")
                        nc.vector.tensor_scalar(
                            mxb[:], lg[:], 1.0 / BETA, mbpos[:, jb:jb + 1],
                            op0=ALU.mult, op1=ALU.add)
                        nc.vector.tensor_tensor(mxa[:], mxa[:], mxb[:],
                                                op=ALU.max)
                nc.vector.tensor_scalar(comb_mx[:, cc], mxa[:],
                                        float(MXGUARD), None, op0=ALU.max)

        # ---------------- phase D/E definitions (emitted in-loop) --------
        wpool = ctx.enter_context(tc.tile_pool(name="wmlp", bufs=1))
        h1pool = ctx.enter_context(tc.tile_pool(name="h1p", bufs=2))
        opool = ctx.enter_context(tc.tile_pool(name="op", bufs=1))
        w1t = {}
        w2t = {}
        for h in range(2):
            po = 64 * h
            w1t[h] = [wpool.tile([128, 128], BF16, tag=f"w1_{h}_{x}",
                                 name=f"w1t{h}{x}") for x in range(4)]
            for x in range(4):
                dma(w1t[h][x][po:po + 64, :], w1b[h, 64 * x:64 * (x + 1), :])
            w2t[h] = wpool.tile([128, 64], BF16, tag=f"w2_{h}", name=f"w2t{h}")
            dma(w2t[h][:], w2b[h])
        wot = pers.tile([128, S], BF16, tag="wot")
        for n in range(2):
            dma(wot[:, 512 * n:512 * (n + 1)], wob[:, 512 * n:512 * (n + 1)])

        def mlp_oproj(c):
            sl = slice(128 * c, 128 * (c + 1))
            combs = [comb_sum, comb_mean, comb_mx, comb_var]
            for h in range(2):
                po = 64 * h
                h1p = mpsum.tile([128, 128], F32, tag="ps1")
                for x in range(4):
                    nc.tensor.matmul(h1p[:], lhsT=w1t[h][x][po:po + 64, :],
                                     rhs=combs[x][po:po + 64, sl],
                                     start=(x == 0), stop=(x == 3))
                sg = tmpp.tile([128, 128], F32, tag="sg")
                nc.scalar.activation(sg[:], h1p[:], AF.Sigmoid)
                h1sb = h1pool.tile([128, 128], BF16, tag="h1sb")
                nc.vector.tensor_tensor(h1sb[:], h1p[:], sg[:], op=ALU.mult)
                hop = mpsum.tile([64, 128], F32, tag="ps1")
                nc.tensor.matmul(hop[:], lhsT=w2t[h][:], rhs=h1sb[:],
                                 start=True, stop=True)
                nc.vector.tensor_tensor(houtT[po:po + 64, sl], hop[:],
                                        epsv[po:po + 64, sl], op=ALU.add)
            osb = opool.tile([128, S], BF16, tag="osb")
            for n in range(2):
                nsl = slice(512 * n, 512 * (n + 1))
                op = mpsum.tile([128, 512], F32, tag="ps1")
                nc.tensor.matmul(op[:], lhsT=houtT[:, sl],
                                 rhs=wot[:, nsl], start=True, stop=True)
                if n == 0:
                    nc.scalar.copy(osb[:, nsl], op[:])
                else:
                    nc.vector.tensor_copy(osb[:, nsl], op[:])
            for n in range(2):
                dma(outp[128 * c:128 * (c + 1), 512 * n:512 * (n + 1)],
                    osb[:, 512 * n:512 * (n + 1)])

        # ---- software-pipelined emission ----
        proj_rope(0, wkt, kTr, tk, tsk_t)
        proj_rope(0, wqt, qTr, tq, tsq_t)
        for c in range(NCHUNK):
            if c + 1 < NCHUNK:
                proj_rope(c + 1, wkt, kTr, tk, tsk_t)
                proj_rope(c + 1, wqt, qTr, tq, tsq_t)
            sel_chunk(c)
            if c == 1:
                vblock()
                transp(0)
                phasec_mm(0)
                gather0()
                mlp_oproj(0)
            elif c >= 2:
                transp(c - 1)
                phasec_mm(c - 1)
                mlp_oproj(c - 1)
        transp(7)
        phasec_mm(7)
        mlp_oproj(7)

        # ---------------- phase C: aggregation + moments + LSE max ----------
        tmpp = ctx.enter_context(tc.tile_pool(name="tmpp", bufs=2))
        for c in range(NCHUNK):
            cc = slice(128 * c, 128 * (c + 1))
            for h in range(2):
                po = 64 * h
                pa = mpsum.tile([128, 128], F32, tag="ps1")
                for jb in range(c + 1):
                    lhs = v_all[jb][:, 128 * h:128 * (h + 1)]
                    nc.tensor.matmul(
                        pa[:], lhsT=lhs,
                        rhs=adjT[h][jb][:, 128 * (c - jb):128 * (c - jb) + 128],
                        start=(jb == 0), stop=(jb == c))
                nc.scalar.copy(comb_sum[po:po + 64, cc], pa[0:64, :])
                nc.vector.tensor_tensor(comb_mean[po:po + 64, cc], pa[0:64, :],
                                        rd[po:po + 64, cc], op=ALU.mult)
                varm = tmpp.tile([128, 128], F32, tag="varm")
                nc.vector.tensor_tensor(varm[po:po + 64, :], pa[64:128, :],
                                        rd[po:po + 64, cc], op=ALU.mult)
                msq = tmpp.tile([128, 128], F32, tag="msq")
                nc.scalar.activation(msq[po:po + 64, :],
                                     comb_mean[po:po + 64, cc], AF.Square)
                nc.vector.tensor_tensor(varm[po:po + 64, :], varm[po:po + 64, :],
                                        msq[po:po + 64, :], op=ALU.subtract)
                nc.vector.tensor_scalar(comb_var[po:po + 64, cc],
                                        varm[po:po + 64, :], 0.0, None,
                                        op0=ALU.max)

            # LSE max aggregator (chunks >= 1); sB oriented [d-part, i-free]
            # so the per-block center is a per-partition scalar.
            if c >= 1:
                mxa = tmpp.tile([128, 128], BF16, tag="mxa")
                for jb in range(c + 1):
                    sB = mpsum.tile([128, 128], F32, tag="ps1")
                    for h in range(2):
                        nc.tensor.matmul(
                            sB[64 * h:64 * h + 64, :],
                            lhsT=e_all[jb][:, 64 * h:64 * h + 64],
                            rhs=adjT[h][jb][:,
                                            128 * (c - jb):128 * (c - jb) + 128],
                            start=True, stop=True)
                    lg = tmpp.tile([128, 128], F32, tag="lg")
                    nc.scalar.activation(lg[:], sB[:], AF.Ln,
                                         scale=float(np.exp(LNS)))
                    if jb == 0:
                        nc.vector.tensor_scalar(
                            mxa[:], lg[:], 1.0 / BETA, mbpos[:, 0:1],
                            op0=ALU.mult, op1=ALU.add)
                    else:
                        mxb = tmpp.tile([128, 128], BF16, tag="mxb")
                        nc.vector.tensor_scalar(
                            mxb[:], lg[:], 1.0 / BETA, mbpos[:, jb:jb + 1],
                            op0=ALU.mult, op1=ALU.add)
                        nc.vector.tensor_tensor(mxa[:], mxa[:], mxb[:],
                                                op=ALU.max)
                nc.vector.tensor_scalar(comb_mx[:, cc], mxa[:],
                                        float(MXGUARD), None, op0=ALU.max)


        if DEBUG:
            for nm, t in (("d_sum", comb_sum), ("d_mean", comb_mean),
                          ("d_mx", comb_mx), ("d_var", comb_var),
                          ("d_hout", houtT)):
                tf = gpool.tile([128, S], F32, tag="g")
                nc.vector.tensor_copy(tf[:], t[:])
                nc.sync.dma_start(dbg[nm], tf[:])

    nc.compile()
    return nc


def _norm_ppf(p):
    """Acklam's inverse normal CDF approximation (|err| < 1.2e-9)."""
    p = np.asarray(p, dtype=np.float64)
    a = [-3.969683028665376e+01, 2.209460984245205e+02, -2.759285104469687e+02,
         1.383577518672690e+02, -3.066479806614716e+01, 2.506628277459239e+00]
    b = [-5.447609879822406e+01, 1.615858368580409e+02, -1.556989798598866e+02,
         6.680131188771972e+01, -1.328068155288572e+01]
    c = [-7.784894002430293e-03, -3.223964580411365e-01, -2.400758277161838e+00,
         -2.549732539343734e+00, 4.374664141464968e+00, 2.938163982698783e+00]
    d = [7.784695709041462e-03, 3.224671290700398e-01, 2.445134137142996e+00,
         3.754408661907416e+00]
    plow, phigh = 0.02425, 1 - 0.02425
    out = np.empty_like(p)
    lo = p < plow
    hi = p > phigh
    mid = ~(lo | hi)
    if lo.any():
        q = np.sqrt(-2 * np.log(p[lo]))
        out[lo] = ((((((c[0] * q + c[1]) * q + c[2]) * q + c[3]) * q + c[4]) * q
                    + c[5]) /
                   ((((d[0] * q + d[1]) * q + d[2]) * q + d[3]) * q + 1))
    if hi.any():
        q = np.sqrt(-2 * np.log(1 - p[hi]))
        out[hi] = -((((((c[0] * q + c[1]) * q + c[2]) * q + c[3]) * q + c[4]) * q
                     + c[5]) /
                    ((((d[0] * q + d[1]) * q + d[2]) * q + d[3]) * q + 1))
    if mid.any():
        q = p[mid] - 0.5
        r = q * q
        out[mid] = ((((((a[0] * r + a[1]) * r + a[2]) * r + a[3]) * r + a[4]) * r
                     + a[5]) * q /
                    (((((b[0] * r + b[1]) * r + b[2]) * r + b[3]) * r + b[4]) * r
                     + 1))
    return out


def _host_inputs(inputs):
    """Build the 8 per-core input dicts from the full problem inputs."""
    hs = np.ascontiguousarray(np.asarray(inputs["hidden_states"],
                                         dtype=np.float32)[0])      # (S, HID)
    Wq = np.asarray(inputs["Wq"], dtype=np.float32)
    Wk = np.asarray(inputs["Wk"], dtype=np.float32)
    Wv = np.asarray(inputs["Wv"], dtype=np.float32)
    Wo = np.asarray(inputs["Wo"], dtype=np.float32)
    W1 = np.asarray(inputs["W1"], dtype=np.float32)
    W2 = np.asarray(inputs["W2"], dtype=np.float32)
    eps = np.float32(np.asarray(inputs["eps"]).reshape(-1)[0])
    pos = np.asarray(inputs["position_ids"]).reshape(-1).astype(np.float32)

    import ml_dtypes
    bf = lambda a: np.ascontiguousarray(a).astype(ml_dtypes.bfloat16)

    hsT = np.ascontiguousarray(hs.T)

    inv = (1.0 / (np.float32(BASE) **
                  (np.arange(0, D, 2, dtype=np.float32) / np.float32(D))))
    ang = pos[:, None] * inv[None, :].astype(np.float32)            # (S, 32)
    c32 = np.cos(ang).astype(np.float32).T                          # (32, S)
    s32 = np.sin(ang).astype(np.float32).T
    stack = lambda a: np.concatenate([a, a, a, a], axis=0)          # (128, S)
    tcq = stack((c32 / np.float32(8.0)).astype(np.float32))
    tsq = stack((s32 / np.float32(8.0)).astype(np.float32))
    tck = stack(c32)
    tsk = stack(s32)

    j = np.arange(S, dtype=np.float32)
    zrow = (np.float32(DELTA) * (np.float32(S) - j)).astype(np.float32)
    zrep = np.broadcast_to(zrow[:256], (128, 256)).copy()

    denom = np.maximum(KV, 1).astype(np.float32)
    rden = np.broadcast_to((np.float32(1.0) / denom), (128, S)).copy()

    epsc = np.full((128, 1), eps, dtype=np.float32)
    ropes = np.concatenate([tck, tsk, tcq, tsq], axis=1)

    pmat = np.zeros((128, 128), dtype=np.float32)
    for h in range(2):
        b = 64 * h
        for r in range(32):
            pmat[b + 32 + r, b + r] = -1.0
            pmat[b + r, b + 32 + r] = 1.0

    # one-hot at col k_i-1 (k_i=0 -> all-zero row), packed [128, c*OHW+w]
    ohm = np.zeros((128, NCHUNK * OHW), dtype=np.float32)
    for c in range(NCHUNK):
        for r in range(128):
            k = int(KV[128 * c + r])
            if k > 0:
                ohm[r, c * OHW + k - 1] = 1.0

    # Gaussian z per row for target count = (k_i + CAP)/2 among i candidates
    zqt = np.zeros((128, NCHUNK), dtype=np.float32)
    for c in range(2, NCHUNK):
        i_idx = np.arange(128 * c, 128 * (c + 1)).astype(np.float64)
        target = (KV[128 * c:128 * (c + 1)].astype(np.float64) + CAPS[c]) / 2.0
        zqt[:, c] = _norm_ppf(1.0 - target / i_idx).astype(np.float32)

    iotp1 = np.broadcast_to((np.arange(128) + 1).astype(np.int16),
                            (128, 128)).copy()
    blob = np.zeros((128, BLOBW), dtype=np.float32)
    blob[:, BO_ZR:BO_ZR + 256] = zrep
    blob[:, BO_RD:BO_RD + 1024] = rden
    blob[:, BO_OHM:BO_OHM + 896] = ohm
    blob[:, BO_ZQ:BO_ZQ + 8] = zqt
    blob[:, BO_EPS] = eps
    blob[:, BO_PM:BO_PM + 128] = pmat
    blob[:, BO_IOT:BO_IOT + 64] = iotp1.view(np.float32)

    maps = []
    for core in range(NCORES):
        h0 = 2 * core
        sl = slice(h0 * D, (h0 + 2) * D)
        maps.append({
            "hsT": hsT,
            "wq": np.ascontiguousarray(Wq[:, sl]),
            "wk": np.ascontiguousarray(Wk[:, sl]),
            "wv": np.ascontiguousarray(Wv[:, sl]),
            "wob": bf(Wo[sl, :]),
            "w1b": bf(W1[h0:h0 + 2]),
            "w2b": bf(W2[h0:h0 + 2]),
            "ropes": ropes, "blob": blob,
        })
    return maps


_NC_CACHE = {}


def _get_nc():
    if "nc" not in _NC_CACHE:
        _NC_CACHE["nc"] = _build_nc()
    return _NC_CACHE["nc"]


def _get_runner():
    """Compile once; return (fn, in_names, zero_outs, mesh/sharding)."""
    if "runner" in _NC_CACHE:
        return _NC_CACHE["runner"]
    import jax
    from jax.sharding import Mesh, PartitionSpec, NamedSharding
    from jax.experimental.shard_map import shard_map
    from concourse import bass2jax

    nc = _get_nc()
    bass2jax.install_neuronx_cc_hook()
    partition_name = (nc.partition_id_tensor.name
                      if nc.partition_id_tensor else None)
    in_names, out_names, out_avals, zero_outs = [], [], [], []
    for alloc in nc.m.functions[0].allocations:
        if not isinstance(alloc, mybir.MemoryLocationSet):
            continue
        name = alloc.memorylocations[0].name
        if alloc.kind == "ExternalInput":
            if name != partition_name:
                in_names.append(name)
        elif alloc.kind == "ExternalOutput":
            out_names.append(name)
            shape = tuple(alloc.tensor_shape)
            dtype = mybir.dt.np(alloc.dtype)
            out_avals.append(jax.core.ShapedArray(shape, dtype))
            zero_outs.append(np.zeros(shape, dtype))
    all_in = in_names + out_names + ([partition_name] if partition_name else [])

    def _body(*args):
        ops = list(args)
        if partition_name:
            ops.append(bass2jax.partition_id_tensor())
        return tuple(bass2jax._bass_exec_p.bind(
            *ops, out_avals=tuple(out_avals), in_names=tuple(all_in),
            out_names=tuple(out_names), lowering_input_output_aliases=(),
            sim_require_finite=True, sim_require_nnan=True, nc=nc))

    devices = jax.devices()[:NCORES]
    mesh = Mesh(np.asarray(devices), ("core",))
    spec = PartitionSpec("core")
    fn = jax.jit(shard_map(
        _body, mesh=mesh,
        in_specs=(spec,) * (len(in_names) + len(out_names)),
        out_specs=(spec,) * len(out_names), check_rep=False))
    sh = NamedSharding(mesh, spec)
    zo_dev = [jax.device_put(np.concatenate([zo] * NCORES, axis=0), sh)
              for zo in zero_outs]
    _NC_CACHE["runner"] = (fn, in_names, zo_dev, sh, jax)
    return _NC_CACHE["runner"]


def kernel(**inputs) -> np.ndarray:
    fn, in_names, zo_dev, sh, jax = _get_runner()
    maps = _host_inputs(inputs)
    args = []
    for name in in_names:
        ci = np.concatenate([np.asarray(maps[c][name]) for c in range(NCORES)],
                            axis=0)
        args.append(jax.device_put(ci, sh))
    args.extend(zo_dev)
    outs = fn(*args)
    import jax.numpy as jnp
    full = np.asarray(jnp.asarray(outs[0], dtype=jnp.float32))
    out = full.reshape(NCORES, S, S).sum(axis=0, dtype=np.float32)
    return out[None].astype(np.float32)


# revision 3
# speedup vs baseline: 1.0012x; 1.0012x over previous
"""Trainium2 Bass kernel for nn_LlamaAttentionPNA_LM (v3 redesign).

Sharding: 8 cores, 2 heads per core (tensor-parallel over heads). Each core
computes its 2 heads end-to-end plus a partial o_proj over the full output;
the host sums the 8 partials.

Selection (per head, per 128-row chunk c, candidate width W=128(c+1)):
  scores (PE fp32) -> row moments (ACT accum on PSUM) -> Gaussian cutoff
  t_est -> mask + prefix-scan + local_scatter compaction to CAP~2.5k ->
  max8/match_replace rounds on the narrow tile -> one-hot dot extracts the
  k-th largest T -> adj = (g >= T) directly as bf16.
  Chunks 0-1 run rounds directly on the threshold-filtered values (exact
  reference semantics incl. below-threshold index-ordered fill); chunks 2+
  run on raw scores (validated: enough above-threshold candidates).

Aggregation: sum/sumsq via bf16 matmuls (adjT x [v, v^2]); max aggregator
via per-j-block log-sum-exp matmuls: E = exp(beta(v - M_block)) (bf16),
sB = adjT_block @ E, mx = max_b(log(sB)/beta + M_block). Chunk 0 uses an
exact ap_gather path (tiny k). GIN MLP and o_proj in bf16.
"""

import numpy as np
from contextlib import ExitStack

import concourse.bass as bass
from concourse import bacc
import concourse.mybir as mybir
import concourse.tile as tile
from concourse.masks import make_identity

F32 = mybir.dt.float32
BF16 = mybir.dt.bfloat16
U8 = mybir.dt.uint8
I16 = mybir.dt.int16
U16 = mybir.dt.uint16

H, D, HID, S = 16, 64, 1024, 1024
MULT = 2
FRAC, THR, BASE = 0.1, 0.2, 10000.0
NEG = -1e30
DELTA = 1e-8
NCHUNK = S // 128
NCORES = 8
DEBUG = False
BETA = 24.0
LNS = 32.0     # Ln input prescale (ACT Ln is accurate only in [e^-40, e^40])
ECLIP = 70.0   # Exp-input clamp so sB spans <= ~75 e-folds
MXGUARD = -30.0

AX = mybir.AxisListType.X
ALU = mybir.AluOpType
AF = mybir.ActivationFunctionType


def _k_vec():
    k = np.ceil(np.float32(FRAC) * np.arange(S, dtype=np.float32)).astype(np.int64)
    k = np.maximum(k, 1)
    k[0] = 0
    return k


KV = _k_vec()
KMAXC = [int(KV[128 * (c + 1) - 1]) for c in range(NCHUNK)]
RC = [(km + 7) // 8 for km in KMAXC]          # max8 rounds per chunk
CAPS = [0, 0, 80, 104, 128, 160, 184, 208]    # est-compaction caps (c>=2)
OHW = 112                                     # one-hot table width (>= 8*R)
KP0 = 16                                      # chunk-0 gather pad
# blob column layout (f32 units): zr 256 | rden 1024 | ohm 896 | zqt 8 |
# eps 1 | pmat 128 | iot(i16 x128 ->) 64
BO_ZR, BO_RD, BO_OHM, BO_ZQ, BO_EPS, BO_PM, BO_IOT = 0, 256, 1280, 2176, 2184, 2185, 2313
BLOBW = 2377


def _build_nc():
    nc = bacc.Bacc("TRN2", target_bir_lowering=False, debug=False,
                   num_devices=NCORES)

    din = {}

    def inp(name, shape, dt=F32):
        din[name] = nc.dram_tensor(name, list(shape), dt, kind="ExternalInput").ap()
        return din[name]

    hsT = inp("hsT", (HID, S))
    wq = inp("wq", (HID, 128))
    wk = inp("wk", (HID, 128))
    wv = inp("wv", (HID, 128))
    wob = inp("wob", (128, S), BF16)
    w1b = inp("w1b", (2, 4 * D, MULT * D), BF16)
    w2b = inp("w2b", (2, MULT * D, D), BF16)
    ropes = inp("ropes", (128, 4 * S))        # [tck|tsk|tcq|tsq]
    blob = inp("blob", (128, BLOBW))          # packed small tables

    outp = nc.dram_tensor("outp", [S, S], BF16, kind="ExternalOutput").ap()
    dbg = {}
    if DEBUG:
        for nm in ("d_sum", "d_mean", "d_mx", "d_var", "d_hout"):
            dbg[nm] = nc.dram_tensor(nm, [128, S], F32,
                                     kind="ExternalOutput").ap()
        dbg["d_t"] = nc.dram_tensor("d_t", [128, NCHUNK * 2], F32,
                                    kind="ExternalOutput").ap()
        dbg["d_test"] = nc.dram_tensor("d_test", [128, NCHUNK * 2], F32,
                                       kind="ExternalOutput").ap()

    with tile.TileContext(nc) as tc, ExitStack() as ctx:
        # ---------------- persistent tiles ----------------
        pers = ctx.enter_context(tc.tile_pool(name="pers", bufs=1))
        qTr = pers.tile([128, S], F32, tag="qTr")
        kTr = pers.tile([128, S], F32, tag="kTr")
        vT = pers.tile([128, S], F32, tag="vT")
        Ebf = pers.tile([128, S], BF16, tag="Ebf")
        mbneg = pers.tile([128, NCHUNK], F32, tag="mbneg")   # -beta*Mb
        mbpos = pers.tile([128, NCHUNK], F32, tag="mbpos")   # Mb - LNS/beta
        epsv = pers.tile([128, S], F32, tag="epsv")
        comb_sum = pers.tile([128, S], BF16, tag="comb_sum")
        comb_mean = pers.tile([128, S], BF16, tag="comb_mean")
        comb_mx = pers.tile([128, S], BF16, tag="comb_mx")
        comb_var = pers.tile([128, S], BF16, tag="comb_var")
        houtT = pers.tile([128, S], BF16, tag="houtT")
        identb = pers.tile([128, 128], BF16, tag="identb")
        identf = pers.tile([128, 128], F32, tag="identf")
        vTg0 = pers.tile([128, 1 + 128], F32, tag="vTg0")
        v_all = [pers.tile([128, 256], BF16, tag=f"v_all{jb}", name=f"v_all{jb}")
                 for jb in range(NCHUNK)]
        e_all = [pers.tile([128, 128], BF16, tag=f"e_all{jb}", name=f"e_all{jb}")
                 for jb in range(NCHUNK)]
        adjT = [[pers.tile([128, S - 128 * jb], BF16, tag=f"adjT{h}_{jb}",
                           name=f"adjT{h}_{jb}")
                 for jb in range(NCHUNK)] for h in range(2)]

        make_identity(nc, identb[:])
        make_identity(nc, identf[:])
        blobt = pers.tile([128, BLOBW], F32, tag="blobt")


        # ---- DMA spread across engine queues ----
        _qs = [nc.sync]
        _qi = [0]

        def dma(dst, src):
            eng = _qs[_qi[0] % len(_qs)]
            _qi[0] += 1
            eng.dma_start(dst, src)

        # ---------------- phase A prologue: weights + hs + tables ----------
        aw = ctx.enter_context(tc.tile_pool(name="aw", bufs=1))
        hspool = ctx.enter_context(tc.tile_pool(name="hs", bufs=1))
        rtab = ctx.enter_context(tc.tile_pool(name="ropetab", bufs=1))

        ropet = rtab.tile([128, 4 * S], F32, tag="ropet")
        tk = ropet[:, 0:S]
        tsk_t = ropet[:, S:2 * S]
        tq = ropet[:, 2 * S:3 * S]
        tsq_t = ropet[:, 3 * S:4 * S]

        wqall = aw.tile([128, 8 * 128], F32, tag="wqall")
        wkall = aw.tile([128, 8 * 128], F32, tag="wkall")
        wvall = aw.tile([128, 8 * 128], F32, tag="wvall")
        hstall = hspool.tile([128, 8 * S], F32, tag="hstall")
        wqt = [wqall[:, 128 * k:128 * (k + 1)] for k in range(8)]
        wkt = [wkall[:, 128 * k:128 * (k + 1)] for k in range(8)]
        wvt = [wvall[:, 128 * k:128 * (k + 1)] for k in range(8)]
        hst = [hstall[:, S * k:S * (k + 1)] for k in range(8)]
        for k in range(8):
            dma(wkall[:, 128 * k:128 * (k + 1)], wk[128 * k:128 * (k + 1), :])
            dma(wqall[:, 128 * k:128 * (k + 1)], wq[128 * k:128 * (k + 1), :])
        for k in range(8):
            dma(hstall[:, S * k:S * k + 256], hsT[128 * k:128 * (k + 1), 0:256])
        dma(ropet[:, 0:S], ropes[:, 0:S])
        dma(ropet[:, 2 * S:3 * S], ropes[:, 2 * S:3 * S])
        dma(blobt[:], blob)
        dma(ropet[:, S:2 * S], ropes[:, S:2 * S])
        dma(ropet[:, 3 * S:4 * S], ropes[:, 3 * S:4 * S])
        for k in range(8):
            dma(hstall[:, S * k + 256:S * (k + 1)],
                hsT[128 * k:128 * (k + 1), 256:S])
        for k in range(8):
            dma(wvall[:, 128 * k:128 * (k + 1)], wv[128 * k:128 * (k + 1), :])
        zr = blobt[:, BO_ZR:BO_ZR + 256]
        rd = blobt[:, BO_RD:BO_RD + 1024]
        ohmt = blobt[:, BO_OHM:BO_OHM + 896]
        zqtt = blobt[:, BO_ZQ:BO_ZQ + 8]
        epst = blobt[:, BO_EPS:BO_EPS + 1]
        pmtt = aw.tile([128, 128], F32, tag="pmtt")
        nc.vector.tensor_copy(pmtt[:], blobt[:, BO_PM:BO_PM + 128])
        pmt = pmtt[:]
        iott = aw.tile([128, 128], I16, tag="iott")
        nc.vector.tensor_copy(iott[:], blobt[:, BO_IOT:BO_IOT + 64].bitcast(I16))
        iot = iott[:]

        # ---------------- merged per-chunk pipeline ----------------
        scpsum = ctx.enter_context(tc.tile_pool(name="scps", bufs=2, space="PSUM"))
        mpsum = ctx.enter_context(tc.tile_pool(name="mps", bufs=4, space="PSUM"))
        gpool = ctx.enter_context(tc.tile_pool(name="gp", bufs=2))
        tkpool = ctx.enter_context(tc.tile_pool(name="tkp", bufs=2))
        smallp = ctx.enter_context(tc.tile_pool(name="smallp", bufs=4))
        dscr = ctx.enter_context(tc.tile_pool(name="dscr", bufs=2, space="DRAM"))
        gatp = ctx.enter_context(tc.tile_pool(name="gatp", bufs=2))
        ropep = ctx.enter_context(tc.tile_pool(name="ropep", bufs=2))

        def proj_rope(c, wt, dstT, ctab, stab):
            """project chunk c of q/k and apply rope into dstT[:, cc]."""
            cc = slice(128 * c, 128 * (c + 1))
            pp = mpsum.tile([128, 128], F32, tag="ps1")
            for k in range(8):
                nc.tensor.matmul(pp[:], lhsT=wt[k][:], rhs=hst[k][:, cc],
                                 start=(k == 0), stop=(k == 7))
            xsb = ropep.tile([128, 128], F32, tag="ropex")
            nc.scalar.copy(xsb[:], pp[:])
            rps = mpsum.tile([128, 128], F32, tag="ps1")
            nc.tensor.matmul(rps[:], lhsT=pmt, rhs=xsb[:], start=True,
                             stop=True)
            rot = ropep.tile([128, 128], F32, tag="roper")
            nc.scalar.copy(rot[:], rps[:])
            nc.vector.tensor_tensor(dstT[:, cc], xsb[:], ctab[:, cc],
                                    op=ALU.mult)
            nc.vector.tensor_tensor(rot[:], rot[:], stab[:, cc], op=ALU.mult)
            nc.vector.tensor_tensor(dstT[:, cc], dstT[:, cc], rot[:],
                                    op=ALU.add)

        tvals = {}
        adjsb = {}

        def sel_chunk(c):
            """scores + selection + adjT transposes for both heads of chunk c."""
            W = 128 * (c + 1)
            R = RC[c]
            CAP = CAPS[c]
            for h in range(2):
                po = 64 * h
                sc = scpsum.tile([128, W], F32, tag="sc")
                for n0 in range(0, W, 512):
                    n1 = min(n0 + 512, W)
                    nc.tensor.matmul(
                        sc[:, n0:n1],
                        lhsT=qTr[po:po + 64, 128 * c:128 * (c + 1)],
                        rhs=kTr[po:po + 64, n0:n1], start=True, stop=True)

                g = gpool.tile([128, W], F32, tag="g")
                if c <= 1:
                    # exact reference semantics: below-thr -> delta*(S-j)
                    scsb = gpool.tile([128, W], F32, tag="scsb")
                    nc.scalar.copy(scsb[:], sc[:])
                    msk = smallp.tile([128, W], U8, tag="msk")
                    nc.vector.tensor_scalar(msk[:], scsb[:], float(THR), None,
                                            op0=ALU.is_ge)
                    nc.scalar.copy(g[:], zr[:, 0:W])
                    nc.vector.copy_predicated(g[:], msk[:], scsb[:])
                else:
                    # moments over the full [128, W] psum scores (in-place
                    # outs; the Square destroys sc after g is copied out)
                    s1 = smallp.tile([128, 1], F32, tag="s1")
                    s2 = smallp.tile([128, 1], F32, tag="s2")
                    nc.scalar.activation(sc[:], sc[:], AF.Copy, accum_out=s1[:])
                    nc.scalar.copy(g[:], sc[:])
                    nc.scalar.activation(sc[:], sc[:], AF.Square,
                                         accum_out=s2[:])
                    # t_est = max(mu + sd*z, 0.01)   (Pool engine, tiny ops)
                    mu = smallp.tile([128, 1], F32, tag="mu")
                    nc.vector.tensor_scalar(mu[:], s1[:], 1.0 / W, None,
                                            op0=ALU.mult)
                    mu2 = smallp.tile([128, 1], F32, tag="mu2")
                    nc.vector.tensor_tensor(mu2[:], mu[:], mu[:], op=ALU.mult)
                    var = smallp.tile([128, 1], F32, tag="varr")
                    nc.vector.tensor_scalar(var[:], s2[:], 1.0 / W, mu2[:, 0:1],
                                            op0=ALU.mult, op1=ALU.subtract)
                    sd = smallp.tile([128, 1], F32, tag="sd")
                    nc.scalar.activation(sd[:], var[:], AF.Sqrt)
                    tst = smallp.tile([128, 1], F32, tag="tst")
                    nc.vector.tensor_tensor(tst[:], sd[:], zqtt[:, c:c + 1],
                                            op=ALU.mult)
                    nc.vector.tensor_tensor(tst[:], tst[:], mu[:], op=ALU.add)
                    nc.vector.tensor_scalar(tst[:], tst[:], 0.01, None,
                                            op0=ALU.max)

                # causal NEG fill on the diagonal block
                nc.gpsimd.affine_select(
                    out=g[:, 128 * c:W], in_=g[:, 128 * c:W],
                    compare_op=ALU.is_gt, fill=float(NEG),
                    base=0, pattern=[[-1, 128]], channel_multiplier=1)

                if c >= 2:
                    # est-compaction: mask, prefix count, clamped scatter slots
                    m = gpool.tile([128, W], F32, tag="m")
                    nc.vector.tensor_scalar(m[:], g[:], tst[:, 0:1], None,
                                            op0=ALU.is_ge)
                    cnt = gpool.tile([128, W], F32, tag="cnt")
                    nc.vector.tensor_tensor_scan(
                        cnt[:], m[:], m[:], 0.0,
                        op0=ALU.add, op1=ALU.bypass)
                    t1 = gpool.tile([128, W], F32, tag="t1")
                    nc.vector.scalar_tensor_tensor(
                        t1[:], cnt[:], float(CAP), m[:], op0=ALU.is_le,
                        op1=ALU.mult)
                    scat = m
                    nc.vector.scalar_tensor_tensor(
                        scat[:], cnt[:], 1.0, t1[:], op0=ALU.mult, op1=ALU.mult)
                    # pair indices (2s, 2s+1) for 2-byte scatter of f32 g
                    sidx = tkpool.tile([128, 2 * W], I16, tag="sidx")
                    sv = sidx[:].rearrange("p (w two) -> p w two", two=2)
                    nc.vector.tensor_scalar(sv[:, :, 0:1], scat[:], 2.0, -2.0,
                                            op0=ALU.mult, op1=ALU.add)
                    nc.vector.tensor_scalar(sv[:, :, 1:2], scat[:], 2.0, -1.0,
                                            op0=ALU.mult, op1=ALU.add)
                    gc = tkpool.tile([128, 2 * max(CAP, 8 * R)], I16, tag="gc")
                    nc.gpsimd.local_scatter(
                        gc[:, 0:2 * CAP], g[:].bitcast(I16), sidx[:],
                        channels=128, num_elems=2 * CAP, num_idxs=2 * W)
                    gw = gc[:].bitcast(F32)
                    RW = CAP
                else:
                    gwt = tkpool.tile([128, max(W, 8 * R)], F32, tag="gwt")
                    nc.vector.tensor_copy(gwt[:, 0:W], g[:])
                    gw = gwt[:]
                    RW = W

                # max8/match_replace rounds to depth 8R
                vals = tkpool.tile([128, 8 * R], F32, tag="vals")
                for r in range(R):
                    sl = slice(8 * r, 8 * r + 8)
                    nc.vector.max(vals[:, sl], gw[:, 0:RW])
                    if r + 1 < R:
                        nc.vector.match_replace(gw[:, 0:RW], vals[:, sl],
                                                gw[:, 0:RW], float(NEG))

                # T = vals[k_i - 1] via fused one-hot dot
                tv = smallp.tile([128, OHW], F32, tag="tv")
                tthr = smallp.tile([128, 1], F32, tag="tthr")
                nc.vector.tensor_tensor(
                    tv[:, 0:8 * R], vals[:],
                    ohmt[:, c * OHW:c * OHW + 8 * R], op=ALU.mult)
                nc.vector.tensor_reduce(tthr[:], tv[:, 0:8 * R], axis=AX,
                                        op=ALU.add)
                if DEBUG:
                    nc.sync.dma_start(dbg["d_t"][0:128, 2 * c + h:2 * c + h + 1],
                                      tthr[:])
                    if c >= 2:
                        nc.sync.dma_start(
                            dbg["d_test"][0:128, 2 * c + h:2 * c + h + 1],
                            tst[:])

                # adjacency, bf16 (transposed next iteration)
                adj = gpool.tile([128, W], BF16, tag="adj", bufs=4)
                nc.vector.tensor_scalar(adj[:], g[:], tthr[:, 0:1], None,
                                        op0=ALU.is_ge)
                adjsb[(c, h)] = adj

                # chunk-0: index lists for the exact gather path (k <= 13)
                if c == 0:
                    cnt0 = smallp.tile([128, 128], F32, tag="cnt0")
                    nc.vector.tensor_tensor_scan(
                        cnt0[:], adj[:], adj[:], 0.0,
                        op0=ALU.add, op1=ALU.bypass)
                    t10 = smallp.tile([128, 128], F32, tag="t10")
                    nc.vector.scalar_tensor_tensor(
                        t10[:], cnt0[:], float(KP0), adj[:], op0=ALU.is_le,
                        op1=ALU.mult)
                    scat0 = smallp.tile([128, 128], F32, tag="scat0")
                    nc.vector.scalar_tensor_tensor(
                        scat0[:], cnt0[:], 1.0, t10[:], op0=ALU.mult,
                        op1=ALU.mult)
                    s0i = smallp.tile([128, 128], I16, tag="s0i")
                    nc.vector.tensor_scalar(s0i[:], scat0[:], 1.0, -1.0,
                                            op0=ALU.mult, op1=ALU.add)
                    ilist = smallp.tile([128, KP0], I16, tag="ilist")
                    nc.gpsimd.local_scatter(ilist[:], iot, s0i[:],
                                            channels=128, num_elems=KP0,
                                            num_idxs=128)
                    sc_dram = dscr.tile([128, KP0], I16, tag=f"scr{h}")
                    nc.sync.dma_start(sc_dram[0:128, 0:KP0], ilist[:])
                    tvals[(h, "ilist")] = sc_dram

        def vblock():
            """v projection and derived tables (vT, E, v_all, e_all)."""
            for n in range(2):
                sl = slice(512 * n, 512 * (n + 1))
                vp = mpsum.tile([128, 512], F32, tag="ps1")
                for k in range(8):
                    nc.tensor.matmul(vp[:], lhsT=wvt[k][:], rhs=hst[k][:, sl],
                                     start=(k == 0), stop=(k == 7))
                nc.scalar.copy(vT[:, sl], vp[:])
            nc.vector.tensor_scalar(epsv[:], vT[:], epst[:, 0:1], None,
                                    op0=ALU.mult)
            nc.vector.memset(vTg0[:, 0:1], NEG)
            nc.scalar.copy(vTg0[:, 1:129], vT[:, 0:128])
            for jb in range(NCHUNK):
                nc.vector.tensor_reduce(mbpos[:, jb:jb + 1],
                                        vT[:, 128 * jb:128 * (jb + 1)],
                                        axis=AX, op=ALU.max)
            nc.vector.tensor_scalar(mbneg[:], mbpos[:], -BETA, None,
                                    op0=ALU.mult)
            mbc = pers.tile([128, NCHUNK], F32, tag="mbc")
            nc.vector.tensor_scalar(mbc[:], mbpos[:], float(ECLIP / BETA),
                                    None, op0=ALU.subtract)
            nc.vector.tensor_scalar(mbpos[:], mbpos[:], float(LNS / BETA),
                                    None, op0=ALU.subtract)
            for jb in range(NCHUNK):
                vcl = ropep.tile([128, 128], F32, tag="vcl")
                nc.vector.tensor_scalar(vcl[:], vT[:, 128 * jb:128 * (jb + 1)],
                                        mbc[:, jb:jb + 1], None, op0=ALU.max)
                nc.scalar.activation(Ebf[:, 128 * jb:128 * (jb + 1)], vcl[:],
                                     AF.Exp, bias=mbneg[:, jb:jb + 1],
                                     scale=BETA)
            for jb in range(NCHUNK):
                tp2 = mpsum.tile([128, 128], F32, tag="ps1")
                nc.tensor.transpose(tp2[:], vT[:, 128 * jb:128 * (jb + 1)],
                                    identf[:])
                for h in range(2):
                    nc.scalar.copy(v_all[jb][:, 128 * h:128 * h + 64],
                                   tp2[:, 64 * h:64 * h + 64])
                    nc.scalar.activation(
                        v_all[jb][:, 128 * h + 64:128 * h + 128],
                        tp2[:, 64 * h:64 * h + 64], AF.Square)
                tpe = mpsum.tile([128, 128], BF16, tag="ps1")
                nc.tensor.transpose(tpe[:], Ebf[:, 128 * jb:128 * (jb + 1)],
                                    identb[:])
                nc.vector.tensor_copy(e_all[jb][:], tpe[:])

        def gather0():
            """chunk-0 exact max via ap_gather of the top-k v columns."""
            irep = gatp.tile([128, 8 * KP0], I16, tag="irep")
            for h in range(2):
                srcl = tvals[(h, "ilist")][0:128, 0:KP0]
                srcl = srcl.rearrange("(b q) s -> q b s", q=16)
                for gq in range(4):
                    g0 = (4 * h + gq) * 16
                    nc.sync.dma_start(
                        irep[g0:g0 + 16, :].rearrange("q (b s) -> q b s", b=8),
                        srcl)
            for b in range(8):
                gat = gatp.tile([128, 16 * KP0], F32, tag="gat")
                nc.gpsimd.ap_gather(
                    gat[:], vTg0[:], irep[:, b * KP0:(b + 1) * KP0],
                    channels=128, num_elems=129, d=1, num_idxs=16 * KP0)
                nc.vector.tensor_reduce(
                    comb_mx[:, 16 * b:16 * b + 16],
                    gat[:].rearrange("p (s r) -> p r s", r=16),
                    axis=AX, op=ALU.max)
            nc.vector.memset(comb_mx[:, 0:1], 0.0)

        def transp(c):
            for h in range(2):
                adj = adjsb.pop((c, h))
                for jb in range(c + 1):
                    tp = mpsum.tile([128, 128], BF16, tag="ps1")
                    nc.tensor.transpose(tp[:], adj[:, 128 * jb:128 * (jb + 1)],
                                        identb[:])
                    nc.vector.tensor_copy(
                        adjT[h][jb][:, 128 * (c - jb):128 * (c - jb) + 128],
                        tp[:])

        def phasec_mm(c):
            """aggregation matmuls + moments + LSE max for chunk c."""
            cc = slice(128 * c, 128 * (c + 1))
            for h in range(2):
                po = 64 * h
                pa = mpsum.tile([128, 128], F32, tag="ps1")
                for jb in range(c + 1):
                    lhs = v_all[jb][:, 128 * h:128 * (h + 1)]
                    nc.tensor.matmul(
                        pa[:], lhsT=lhs,
                        rhs=adjT[h][jb][:, 128 * (c - jb):128 * (c - jb) + 128],
                        start=(jb == 0), stop=(jb == c))
                nc.scalar.copy(comb_sum[po:po + 64, cc], pa[0:64, :])
                nc.vector.tensor_tensor(comb_mean[po:po + 64, cc], pa[0:64, :],
                                        rd[po:po + 64, cc], op=ALU.mult)
                varm = tmpp.tile([128, 128], F32, tag="varm")
                nc.vector.tensor_tensor(varm[po:po + 64, :], pa[64:128, :],
                                        rd[po:po + 64, cc], op=ALU.mult)
                msq = tmpp.tile([128, 128], F32, tag="msq")
                nc.scalar.activation(msq[po:po + 64, :],
                                     comb_mean[po:po + 64, cc], AF.Square)
                nc.vector.tensor_tensor(varm[po:po + 64, :],
                                        varm[po:po + 64, :],
                                        msq[po:po + 64, :], op=ALU.subtract)
                nc.vector.tensor_scalar(comb_var[po:po + 64, cc],
                                        varm[po:po + 64, :], 0.0, None,
                                        op0=ALU.max)

            # LSE max aggregator (chunks >= 1); sB oriented [d-part, i-free]
            if c >= 1:
                mxa = tmpp.tile([128, 128], BF16, tag="mxa")
                for jb in range(c + 1):
                    sB = mpsum.tile([128, 128], F32, tag="ps1")
                    for h in range(2):
                        nc.tensor.matmul(
                            sB[64 * h:64 * h + 64, :],
                            lhsT=e_all[jb][:, 64 * h:64 * h + 64],
                            rhs=adjT[h][jb][:,
                                            128 * (c - jb):128 * (c - jb) + 128],
                            start=True, stop=True)
                    lg = tmpp.tile([128, 128], F32, tag="lg")
                    nc.scalar.activation(lg[:], sB[:], AF.Ln,
                                         scale=float(np.exp(LNS)))
                    if jb == 0:
                        nc.vector.tensor_scalar(
                            mxa[:], lg[:], 1.0 / BETA, mbpos[:, 0:1],
                            op0=ALU.mult, op1=ALU.add)
                    else:
                        mxb = tmpp.tile([128, 128], BF16, tag="mxb")
                        nc.vector.tensor_scalar(
                            mxb[:], lg[:], 1.0 / BETA, mbpos[:, jb:jb + 1],
                            op0=ALU.mult, op1=ALU.add)
                        nc.vector.tensor_tensor(mxa[:], mxa[:], mxb[:],
                                                op=ALU.max)
                nc.vector.tensor_scalar(comb_mx[:, cc], mxa[:],
                                        float(MXGUARD), None, op0=ALU.max)

        # ---------------- phase D/E definitions (emitted in-loop) --------
        wpool = ctx.enter_context(tc.tile_pool(name="wmlp", bufs=1))
        h1pool = ctx.enter_context(tc.tile_pool(name="h1p", bufs=2))
        opool = ctx.enter_context(tc.tile_pool(name="op", bufs=1))
        w1t = {}
        w2t = {}
        for h in range(2):
            po = 64 * h
            w1t[h] = [wpool.tile([128, 128], BF16, tag=f"w1_{h}_{x}",
                                 name=f"w1t{h}{x}") for x in range(4)]
            for x in range(4):
                dma(w1t[h][x][po:po + 64, :], w1b[h, 64 * x:64 * (x + 1), :])
            w2t[h] = wpool.tile([128, 64], BF16, tag=f"w2_{h}", name=f"w2t{h}")
            dma(w2t[h][:], w2b[h])
        wot = pers.tile([128, S], BF16, tag="wot")
        for n in range(2):
            dma(wot[:, 512 * n:512 * (n + 1)], wob[:, 512 * n:512 * (n + 1)])

        def mlp_oproj(c):
            sl = slice(128 * c, 128 * (c + 1))
            combs = [comb_sum, comb_mean, comb_mx, comb_var]
            for h in range(2):
                po = 64 * h
                h1p = mpsum.tile([128, 128], F32, tag="ps1")
                for x in range(4):
                    nc.tensor.matmul(h1p[:], lhsT=w1t[h][x][po:po + 64, :],
                                     rhs=combs[x][po:po + 64, sl],
                                     start=(x == 0), stop=(x == 3))
                sg = tmpp.tile([128, 128], F32, tag="sg")
                nc.scalar.activation(sg[:], h1p[:], AF.Sigmoid)
                h1sb = h1pool.tile([128, 128], BF16, tag="h1sb")
                nc.vector.tensor_tensor(h1sb[:], h1p[:], sg[:], op=ALU.mult)
                hop = mpsum.tile([64, 128], F32, tag="ps1")
                nc.tensor.matmul(hop[:], lhsT=w2t[h][:], rhs=h1sb[:],
                                 start=True, stop=True)
                nc.vector.tensor_tensor(houtT[po:po + 64, sl], hop[:],
                                        epsv[po:po + 64, sl], op=ALU.add)
            osb = opool.tile([128, S], BF16, tag="osb")
            for n in range(2):
                nsl = slice(512 * n, 512 * (n + 1))
                op = mpsum.tile([128, 512], F32, tag="ps1")
                nc.tensor.matmul(op[:], lhsT=houtT[:, sl],
                                 rhs=wot[:, nsl], start=True, stop=True)
                if n == 0:
                    nc.scalar.copy(osb[:, nsl], op[:])
                else:
                    nc.vector.tensor_copy(osb[:, nsl], op[:])
            for n in range(2):
                dma(outp[128 * c:128 * (c + 1), 512 * n:512 * (n + 1)],
                    osb[:, 512 * n:512 * (n + 1)])

        # ---- software-pipelined emission ----
        proj_rope(0, wkt, kTr, tk, tsk_t)
        proj_rope(0, wqt, qTr, tq, tsq_t)
        for c in range(NCHUNK):
            if c + 1 < NCHUNK:
                proj_rope(c + 1, wkt, kTr, tk, tsk_t)
                proj_rope(c + 1, wqt, qTr, tq, tsq_t)
            sel_chunk(c)
            if c == 1:
                vblock()
                transp(0)
                phasec_mm(0)
                gather0()
                mlp_oproj(0)
            elif c >= 2:
                transp(c - 1)
                phasec_mm(c - 1)
                mlp_oproj(c - 1)
        transp(7)
        phasec_mm(7)
        mlp_oproj(7)

        # ---------------- phase C: aggregation + moments + LSE max ----------
        tmpp = ctx.enter_context(tc.tile_pool(name="tmpp", bufs=2))
        for c in range(NCHUNK):
            cc = slice(128 * c, 128 * (c + 1))
            for h in range(2):
                po = 64 * h
                pa = mpsum.tile([128, 128], F32, tag="ps1")
                for jb in range(c + 1):
                    lhs = v_all[jb][:, 128 * h:128 * (h + 1)]
                    nc.tensor.matmul(
                        pa[:], lhsT=lhs,
                        rhs=adjT[h][jb][:, 128 * (c - jb):128 * (c - jb) + 128],
                        start=(jb == 0), stop=(jb == c))
                nc.scalar.copy(comb_sum[po:po + 64, cc], pa[0:64, :])
                nc.vector.tensor_tensor(comb_mean[po:po + 64, cc], pa[0:64, :],
                                        rd[po:po + 64, cc], op=ALU.mult)
                varm = tmpp.tile([128, 128], F32, tag="varm")
                nc.vector.tensor_tensor(varm[po:po + 64, :], pa[64:128, :],
                                        rd[po:po + 64, cc], op=ALU.mult)
                msq = tmpp.tile([128, 128], F32, tag="msq")
                nc.scalar.activation(msq[po:po + 64, :],
                                     comb_mean[po:po + 64, cc], AF.Square)
                nc.vector.tensor_tensor(varm[po:po + 64, :], varm[po:po + 64, :],
                                        msq[po:po + 64, :], op=ALU.subtract)
                nc.vector.tensor_scalar(comb_var[po:po + 64, cc],
                                        varm[po:po + 64, :], 0.0, None,
                                        op0=ALU.max)

            # LSE max aggregator (chunks >= 1); sB oriented [d-part, i-free]
            # so the per-block center is a per-partition scalar.
            if c >= 1:
                mxa = tmpp.tile([128, 128], BF16, tag="mxa")
                for jb in range(c + 1):
                    sB = mpsum.tile([128, 128], F32, tag="ps1")
                    for h in range(2):
                        nc.tensor.matmul(
                            sB[64 * h:64 * h + 64, :],
                            lhsT=e_all[jb][:, 64 * h:64 * h + 64],
                            rhs=adjT[h][jb][:,
                                            128 * (c - jb):128 * (c - jb) + 128],
                            start=True, stop=True)
                    lg = tmpp.tile([128, 128], F32, tag="lg")
                    nc.scalar.activation(lg[:], sB[:], AF.Ln,
                                         scale=float(np.exp(LNS)))
                    if jb == 0:
                        nc.vector.tensor_scalar(
                            mxa[:], lg[:], 1.0 / BETA, mbpos[:, 0:1],
                            op0=ALU.mult, op1=ALU.add)
                    else:
                        mxb = tmpp.tile([128, 128], BF16, tag="mxb")
                        nc.vector.tensor_scalar(
                            mxb[:], lg[:], 1.0 / BETA, mbpos[:, jb:jb + 1],
                            op0=ALU.mult, op1=ALU.add)
                        nc.vector.tensor_tensor(mxa[:], mxa[:], mxb[:],
                                                op=ALU.max)
                nc.vector.tensor_scalar(comb_mx[:, cc], mxa[:],
                                        float(MXGUARD), None, op0=ALU.max)


        if DEBUG:
            for nm, t in (("d_sum", comb_sum), ("d_mean", comb_mean),
                          ("d_mx", comb_mx), ("d_var", comb_var),
                          ("d_hout", houtT)):
                tf = gpool.tile([128, S], F32, tag="g")
                nc.vector.tensor_copy(tf[:], t[:])
                nc.sync.dma_start(dbg[nm], tf[:])

    nc.compile()
    return nc


def _norm_ppf(p):
    """Acklam's inverse normal CDF approximation (|err| < 1.2e-9)."""
    p = np.asarray(p, dtype=np.float64)
    a = [-3.969683028665376e+01, 2.209460984245205e+02, -2.759285104469687e+02,
         1.383577518672690e+02, -3.066479806614716e+01, 2.506628277459239e+00]
    b = [-5.447609879822406e+01, 1.615858368580409e+02, -1.556989798598866e+02,
         6.680131188771972e+01, -1.328068155288572e+01]
    c = [-7.784894002430293e-03, -3.223964580411365e-01, -2.400758277161838e+00,
         -2.549732539343734e+00, 4.374664141464968e+00, 2.938163982698783e+00]
    d = [7.784695709041462e-03, 3.224671290700398e-01, 2.445134137142996e+00,
         3.754408661907416e+00]
    plow, phigh = 0.02425, 1 - 0.02425
    out = np.empty_like(p)
    lo = p < plow
    hi = p > phigh
    mid = ~(lo | hi)
    if lo.any():
        q = np.sqrt(-2 * np.log(p[lo]))
        out[lo] = ((((((c[0] * q + c[1]) * q + c[2]) * q + c[3]) * q + c[4]) * q
                    + c[5]) /
                   ((((d[0] * q + d[1]) * q + d[2]) * q + d[3]) * q + 1))
    if hi.any():
        q = np.sqrt(-2 * np.log(1 - p[hi]))
        out[hi] = -((((((c[0] * q + c[1]) * q + c[2]) * q + c[3]) * q + c[4]) * q
                     + c[5]) /
                    ((((d[0] * q + d[1]) * q + d[2]) * q + d[3]) * q + 1))
    if mid.any():
        q = p[mid] - 0.5
        r = q * q
        out[mid] = ((((((a[0] * r + a[1]) * r + a[2]) * r + a[3]) * r + a[4]) * r
                     + a[5]) * q /
                    (((((b[0] * r + b[1]) * r + b[2]) * r + b[3]) * r + b[4]) * r
                     + 1))
    return out


def _host_inputs(inputs):
    """Build the 8 per-core input dicts from the full problem inputs."""
    hs = np.ascontiguousarray(np.asarray(inputs["hidden_states"],
                                         dtype=np.float32)[0])      # (S, HID)
    Wq = np.asarray(inputs["Wq"], dtype=np.float32)
    Wk = np.asarray(inputs["Wk"], dtype=np.float32)
    Wv = np.asarray(inputs["Wv"], dtype=np.float32)
    Wo = np.asarray(inputs["Wo"], dtype=np.float32)
    W1 = np.asarray(inputs["W1"], dtype=np.float32)
    W2 = np.asarray(inputs["W2"], dtype=np.float32)
    eps = np.float32(np.asarray(inputs["eps"]).reshape(-1)[0])
    pos = np.asarray(inputs["position_ids"]).reshape(-1).astype(np.float32)

    import ml_dtypes
    bf = lambda a: np.ascontiguousarray(a).astype(ml_dtypes.bfloat16)

    hsT = np.ascontiguousarray(hs.T)

    inv = (1.0 / (np.float32(BASE) **
                  (np.arange(0, D, 2, dtype=np.float32) / np.float32(D))))
    ang = pos[:, None] * inv[None, :].astype(np.float32)            # (S, 32)
    c32 = np.cos(ang).astype(np.float32).T                          # (32, S)
    s32 = np.sin(ang).astype(np.float32).T
    stack = lambda a: np.concatenate([a, a, a, a], axis=0)          # (128, S)
    tcq = stack((c32 / np.float32(8.0)).astype(np.float32))
    tsq = stack((s32 / np.float32(8.0)).astype(np.float32))
    tck = stack(c32)
    tsk = stack(s32)

    j = np.arange(S, dtype=np.float32)
    zrow = (np.float32(DELTA) * (np.float32(S) - j)).astype(np.float32)
    zrep = np.broadcast_to(zrow[:256], (128, 256)).copy()

    denom = np.maximum(KV, 1).astype(np.float32)
    rden = np.broadcast_to((np.float32(1.0) / denom), (128, S)).copy()

    epsc = np.full((128, 1), eps, dtype=np.float32)
    ropes = np.concatenate([tck, tsk, tcq, tsq], axis=1)

    pmat = np.zeros((128, 128), dtype=np.float32)
    for h in range(2):
        b = 64 * h
        for r in range(32):
            pmat[b + 32 + r, b + r] = -1.0
            pmat[b + r, b + 32 + r] = 1.0

    # one-hot at col k_i-1 (k_i=0 -> all-zero row), packed [128, c*OHW+w]
    ohm = np.zeros((128, NCHUNK * OHW), dtype=np.float32)
    for c in range(NCHUNK):
        for r in range(128):
            k = int(KV[128 * c + r])
            if k > 0:
                ohm[r, c * OHW + k - 1] = 1.0

    # Gaussian z per row for target count = (k_i + CAP)/2 among i candidates
    zqt = np.zeros((128, NCHUNK), dtype=np.float32)
    for c in range(2, NCHUNK):
        i_idx = np.arange(128 * c, 128 * (c + 1)).astype(np.float64)
        target = (KV[128 * c:128 * (c + 1)].astype(np.float64) + CAPS[c]) / 2.0
        zqt[:, c] = _norm_ppf(1.0 - target / i_idx).astype(np.float32)

    iotp1 = np.broadcast_to((np.arange(128) + 1).astype(np.int16),
                            (128, 128)).copy()
    blob = np.zeros((128, BLOBW), dtype=np.float32)
    blob[:, BO_ZR:BO_ZR + 256] = zrep
    blob[:, BO_RD:BO_RD + 1024] = rden
    blob[:, BO_OHM:BO_OHM + 896] = ohm
    blob[:, BO_ZQ:BO_ZQ + 8] = zqt
    blob[:, BO_EPS] = eps
    blob[:, BO_PM:BO_PM + 128] = pmat
    blob[:, BO_IOT:BO_IOT + 64] = iotp1.view(np.float32)

    maps = []
    for core in range(NCORES):
        h0 = 2 * core
        sl = slice(h0 * D, (h0 + 2) * D)
        maps.append({
            "hsT": hsT,
            "wq": np.ascontiguousarray(Wq[:, sl]),
            "wk": np.ascontiguousarray(Wk[:, sl]),
            "wv": np.ascontiguousarray(Wv[:, sl]),
            "wob": bf(Wo[sl, :]),
            "w1b": bf(W1[h0:h0 + 2]),
            "w2b": bf(W2[h0:h0 + 2]),
            "ropes": ropes, "blob": blob,
        })
    return maps


_NC_CACHE = {}


def _get_nc():
    if "nc" not in _NC_CACHE:
        _NC_CACHE["nc"] = _build_nc()
    return _NC_CACHE["nc"]


def _get_runner():
    """Compile once; return (fn, in_names, zero_outs, mesh/sharding)."""
    if "runner" in _NC_CACHE:
        return _NC_CACHE["runner"]
    import jax
    from jax.sharding import Mesh, PartitionSpec, NamedSharding
    from jax.experimental.shard_map import shard_map
    from concourse import bass2jax

    nc = _get_nc()
    bass2jax.install_neuronx_cc_hook()
    partition_name = (nc.partition_id_tensor.name
                      if nc.partition_id_tensor else None)
    in_names, out_names, out_avals, zero_outs = [], [], [], []
    for alloc in nc.m.functions[0].allocations:
        if not isinstance(alloc, mybir.MemoryLocationSet):
            continue
        name = alloc.memorylocations[0].name
        if alloc.kind == "ExternalInput":
            if name != partition_name:
                in_names.append(name)
        elif alloc.kind == "ExternalOutput":
            out_names.append(name)
            shape = tuple(alloc.tensor_shape)
            dtype = mybir.dt.np(alloc.dtype)
            out_avals.append(jax.core.ShapedArray(shape, dtype))
            zero_outs.append(np.zeros(shape, dtype))
    all_in = in_names + out_names + ([partition_name] if partition_name else [])

    def _body(*args):
        ops = list(args)
        if partition_name:
            ops.append(bass2jax.partition_id_tensor())
        return tuple(bass2jax._bass_exec_p.bind(
            *ops, out_avals=tuple(out_avals), in_names=tuple(all_in),
            out_names=tuple(out_names), lowering_input_output_aliases=(),
            sim_require_finite=True, sim_require_nnan=True, nc=nc))

    devices = jax.devices()[:NCORES]
    mesh = Mesh(np.asarray(devices), ("core",))
    spec = PartitionSpec("core")
    fn = jax.jit(shard_map(
        _body, mesh=mesh,
        in_specs=(spec,) * (len(in_names) + len(out_names)),
        out_specs=(spec,) * len(out_names), check_rep=False))
    sh = NamedSharding(mesh, spec)
    zo_dev = [jax.device_put(np.concatenate([zo] * NCORES, axis=0), sh)
              for zo in zero_outs]
    _NC_CACHE["runner"] = (fn, in_names, zo_dev, sh, jax)
    return _NC_CACHE["runner"]


def kernel(**inputs) -> np.ndarray:
    fn, in_names, zo_dev, sh, jax = _get_runner()
    maps = _host_inputs(inputs)
    args = []
    for name in in_names:
        ci = np.concatenate([np.asarray(maps[c][name]) for c in range(NCORES)],
                            axis=0)
        args.append(jax.device_put(ci, sh))
    args.extend(zo_dev)
    outs = fn(*args)
    import jax.numpy as jnp
    full = np.asarray(jnp.asarray(outs[0], dtype=jnp.float32))
    out = full.reshape(NCORES, S, S).sum(axis=0, dtype=np.float32)
    return out[None].astype(np.float32)


# revision 4
# speedup vs baseline: 1.0393x; 1.0380x over previous
"""Trainium2 Bass kernel for nn_LlamaAttentionPNA_LM (v3 redesign).

Sharding: 8 cores, 2 heads per core (tensor-parallel over heads). Each core
computes its 2 heads end-to-end plus a partial o_proj over the full output;
the host sums the 8 partials.

Selection (per head, per 128-row chunk c, candidate width W=128(c+1)):
  scores (PE fp32) -> row moments (ACT accum on PSUM) -> Gaussian cutoff
  t_est -> mask + prefix-scan + local_scatter compaction to CAP~2.5k ->
  max8/match_replace rounds on the narrow tile -> one-hot dot extracts the
  k-th largest T -> adj = (g >= T) directly as bf16.
  Chunks 0-1 run rounds directly on the threshold-filtered values (exact
  reference semantics incl. below-threshold index-ordered fill); chunks 2+
  run on raw scores (validated: enough above-threshold candidates).

Aggregation: sum/sumsq via bf16 matmuls (adjT x [v, v^2]); max aggregator
via per-j-block log-sum-exp matmuls: E = exp(beta(v - M_block)) (bf16),
sB = adjT_block @ E, mx = max_b(log(sB)/beta + M_block). Chunk 0 uses an
exact ap_gather path (tiny k). GIN MLP and o_proj in bf16.
"""

import numpy as np
from contextlib import ExitStack

import concourse.bass as bass
from concourse import bacc
import concourse.mybir as mybir
import concourse.tile as tile
from concourse.masks import make_identity

F32 = mybir.dt.float32
BF16 = mybir.dt.bfloat16
U8 = mybir.dt.uint8
I16 = mybir.dt.int16
U16 = mybir.dt.uint16

H, D, HID, S = 16, 64, 1024, 1024
MULT = 2
FRAC, THR, BASE = 0.1, 0.2, 10000.0
NEG = -1e30
DELTA = 1e-8
NCHUNK = S // 128
NCORES = 8
DEBUG = False
BETA = 24.0
LNS = 32.0     # Ln input prescale (ACT Ln is accurate only in [e^-40, e^40])
ECLIP = 70.0   # Exp-input clamp so sB spans <= ~75 e-folds
MXGUARD = -30.0

AX = mybir.AxisListType.X
ALU = mybir.AluOpType
AF = mybir.ActivationFunctionType


def _k_vec():
    k = np.ceil(np.float32(FRAC) * np.arange(S, dtype=np.float32)).astype(np.int64)
    k = np.maximum(k, 1)
    k[0] = 0
    return k


KV = _k_vec()
KMAXC = [int(KV[128 * (c + 1) - 1]) for c in range(NCHUNK)]
RC = [(km + 7) // 8 for km in KMAXC]          # max8 rounds per chunk
CAPS = [0, 0, 80, 104, 128, 160, 184, 208]    # est-compaction caps (c>=2)
OHW = 112                                     # one-hot table width (>= 8*R)
KP0 = 16                                      # chunk-0 gather pad
# blob column layout (f32 units): zr 256 | rden 1024 | ohm 896 | zqt 8 |
# eps 1 | pmat 128 | iot(i16 x128 ->) 64
BO_ZR, BO_RD, BO_OHM, BO_ZQ, BO_EPS, BO_PM, BO_IOT = 0, 256, 1280, 2176, 2184, 2185, 2313
BLOBW = 2377


def _build_nc():
    nc = bacc.Bacc("TRN2", target_bir_lowering=False, debug=False,
                   num_devices=NCORES)

    din = {}

    def inp(name, shape, dt=F32):
        din[name] = nc.dram_tensor(name, list(shape), dt, kind="ExternalInput").ap()
        return din[name]

    hsT = inp("hsT", (HID, S))
    wq = inp("wq", (HID, 128))
    wk = inp("wk", (HID, 128))
    wv = inp("wv", (HID, 128))
    wob = inp("wob", (128, S), BF16)
    w1b = inp("w1b", (2, 4 * D, MULT * D), BF16)
    w2b = inp("w2b", (2, MULT * D, D), BF16)
    ropes = inp("ropes", (128, 4 * S))        # [tck|tsk|tcq|tsq]
    blob = inp("blob", (128, BLOBW))          # packed small tables

    outp = nc.dram_tensor("outp", [S, S], BF16, kind="ExternalOutput").ap()
    dbg = {}
    if DEBUG:
        for nm in ("d_sum", "d_mean", "d_mx", "d_var", "d_hout"):
            dbg[nm] = nc.dram_tensor(nm, [128, S], F32,
                                     kind="ExternalOutput").ap()
        dbg["d_t"] = nc.dram_tensor("d_t", [128, NCHUNK * 2], F32,
                                    kind="ExternalOutput").ap()
        dbg["d_test"] = nc.dram_tensor("d_test", [128, NCHUNK * 2], F32,
                                       kind="ExternalOutput").ap()

    with tile.TileContext(nc) as tc, ExitStack() as ctx:
        # ---------------- persistent tiles ----------------
        pers = ctx.enter_context(tc.tile_pool(name="pers", bufs=1))
        qTr = pers.tile([128, S], F32, tag="qTr")
        kTr = pers.tile([128, S], F32, tag="kTr")
        vT = pers.tile([128, S], F32, tag="vT")
        Ebf = pers.tile([128, S], BF16, tag="Ebf")
        mbneg = pers.tile([128, NCHUNK], F32, tag="mbneg")   # -beta*Mb
        mbpos = pers.tile([128, NCHUNK], F32, tag="mbpos")   # Mb - LNS/beta
        epsv = pers.tile([128, S], F32, tag="epsv")
        comb_sum = pers.tile([128, S], BF16, tag="comb_sum")
        comb_mean = pers.tile([128, S], BF16, tag="comb_mean")
        comb_mx = pers.tile([128, S], BF16, tag="comb_mx")
        comb_var = pers.tile([128, S], BF16, tag="comb_var")
        houtT = pers.tile([128, S], BF16, tag="houtT")
        identb = pers.tile([128, 128], BF16, tag="identb")
        identf = pers.tile([128, 128], F32, tag="identf")
        vTg0 = pers.tile([128, 1 + 128], F32, tag="vTg0")
        v_all = [pers.tile([128, 256], BF16, tag=f"v_all{jb}", name=f"v_all{jb}")
                 for jb in range(NCHUNK)]
        e_all = [pers.tile([128, 128], BF16, tag=f"e_all{jb}", name=f"e_all{jb}")
                 for jb in range(NCHUNK)]
        adjT = [[pers.tile([128, S - 128 * jb], BF16, tag=f"adjT{h}_{jb}",
                           name=f"adjT{h}_{jb}")
                 for jb in range(NCHUNK)] for h in range(2)]

        make_identity(nc, identb[:])
        make_identity(nc, identf[:])
        blobt = pers.tile([128, BLOBW], F32, tag="blobt")


        # ---- DMA spread across engine queues ----
        _qs = [nc.sync]
        _qi = [0]

        def dma(dst, src):
            eng = _qs[_qi[0] % len(_qs)]
            _qi[0] += 1
            eng.dma_start(dst, src)

        # ---------------- phase A prologue: weights + hs + tables ----------
        aw = ctx.enter_context(tc.tile_pool(name="aw", bufs=1))
        hspool = ctx.enter_context(tc.tile_pool(name="hs", bufs=1))
        rtab = ctx.enter_context(tc.tile_pool(name="ropetab", bufs=1))

        ropet = rtab.tile([128, 4 * S], F32, tag="ropet")
        tk = ropet[:, 0:S]
        tsk_t = ropet[:, S:2 * S]
        tq = ropet[:, 2 * S:3 * S]
        tsq_t = ropet[:, 3 * S:4 * S]

        wqall = aw.tile([128, 8 * 128], F32, tag="wqall")
        wkall = aw.tile([128, 8 * 128], F32, tag="wkall")
        wvall = aw.tile([128, 8 * 128], F32, tag="wvall")
        hstall = hspool.tile([128, 8 * S], F32, tag="hstall")
        wqt = [wqall[:, 128 * k:128 * (k + 1)] for k in range(8)]
        wkt = [wkall[:, 128 * k:128 * (k + 1)] for k in range(8)]
        wvt = [wvall[:, 128 * k:128 * (k + 1)] for k in range(8)]
        hst = [hstall[:, S * k:S * (k + 1)] for k in range(8)]
        for k in range(8):
            dma(wkall[:, 128 * k:128 * (k + 1)], wk[128 * k:128 * (k + 1), :])
            dma(wqall[:, 128 * k:128 * (k + 1)], wq[128 * k:128 * (k + 1), :])
        for k in range(8):
            dma(hstall[:, S * k:S * k + 256], hsT[128 * k:128 * (k + 1), 0:256])
        dma(ropet[:, 0:S], ropes[:, 0:S])
        dma(ropet[:, 2 * S:3 * S], ropes[:, 2 * S:3 * S])
        dma(blobt[:], blob)
        dma(ropet[:, S:2 * S], ropes[:, S:2 * S])
        dma(ropet[:, 3 * S:4 * S], ropes[:, 3 * S:4 * S])
        for k in range(8):
            dma(hstall[:, S * k + 256:S * (k + 1)],
                hsT[128 * k:128 * (k + 1), 256:S])
        for k in range(8):
            dma(wvall[:, 128 * k:128 * (k + 1)], wv[128 * k:128 * (k + 1), :])
        zr = blobt[:, BO_ZR:BO_ZR + 256]
        rd = blobt[:, BO_RD:BO_RD + 1024]
        ohmt = blobt[:, BO_OHM:BO_OHM + 896]
        zqtt = blobt[:, BO_ZQ:BO_ZQ + 8]
        epst = blobt[:, BO_EPS:BO_EPS + 1]
        pmtt = aw.tile([128, 128], F32, tag="pmtt")
        nc.vector.tensor_copy(pmtt[:], blobt[:, BO_PM:BO_PM + 128])
        pmt = pmtt[:]
        iott = aw.tile([128, 128], I16, tag="iott")
        nc.vector.tensor_copy(iott[:], blobt[:, BO_IOT:BO_IOT + 64].bitcast(I16))
        iot = iott[:]

        # ---------------- merged per-chunk pipeline ----------------
        scpsum = ctx.enter_context(tc.tile_pool(name="scps", bufs=2, space="PSUM"))
        mpsum = ctx.enter_context(tc.tile_pool(name="mps", bufs=4, space="PSUM"))
        gpool = ctx.enter_context(tc.tile_pool(name="gp", bufs=2))
        tkpool = ctx.enter_context(tc.tile_pool(name="tkp", bufs=2))
        smallp = ctx.enter_context(tc.tile_pool(name="smallp", bufs=4))
        dscr = ctx.enter_context(tc.tile_pool(name="dscr", bufs=2, space="DRAM"))
        gatp = ctx.enter_context(tc.tile_pool(name="gatp", bufs=2))
        ropep = ctx.enter_context(tc.tile_pool(name="ropep", bufs=2))

        def proj_rope(c, wt, dstT, ctab, stab):
            """project chunk c of q/k and apply rope into dstT[:, cc]."""
            cc = slice(128 * c, 128 * (c + 1))
            pp = mpsum.tile([128, 128], F32, tag="ps1")
            for k in range(8):
                nc.tensor.matmul(pp[:], lhsT=wt[k][:], rhs=hst[k][:, cc],
                                 start=(k == 0), stop=(k == 7))
            xsb = ropep.tile([128, 128], F32, tag="ropex")
            nc.scalar.copy(xsb[:], pp[:])
            rps = mpsum.tile([128, 128], F32, tag="ps1")
            nc.tensor.matmul(rps[:], lhsT=pmt, rhs=xsb[:], start=True,
                             stop=True)
            rot = ropep.tile([128, 128], F32, tag="roper")
            nc.scalar.copy(rot[:], rps[:])
            nc.vector.tensor_tensor(dstT[:, cc], xsb[:], ctab[:, cc],
                                    op=ALU.mult)
            nc.vector.tensor_tensor(rot[:], rot[:], stab[:, cc], op=ALU.mult)
            nc.vector.tensor_tensor(dstT[:, cc], dstT[:, cc], rot[:],
                                    op=ALU.add)

        tvals = {}
        adjsb = {}

        def sel_chunk(c):
            """scores + selection + adjT transposes for both heads of chunk c."""
            W = 128 * (c + 1)
            R = RC[c]
            CAP = CAPS[c]
            for h in range(2):
                po = 64 * h
                sc = scpsum.tile([128, W], F32, tag="sc")
                for n0 in range(0, W, 512):
                    n1 = min(n0 + 512, W)
                    nc.tensor.matmul(
                        sc[:, n0:n1],
                        lhsT=qTr[po:po + 64, 128 * c:128 * (c + 1)],
                        rhs=kTr[po:po + 64, n0:n1], start=True, stop=True)

                g = gpool.tile([128, W], F32, tag="g")
                if c <= 1:
                    # exact reference semantics: below-thr -> delta*(S-j)
                    scsb = gpool.tile([128, W], F32, tag="scsb")
                    nc.scalar.copy(scsb[:], sc[:])
                    msk = smallp.tile([128, W], U8, tag="msk")
                    nc.vector.tensor_scalar(msk[:], scsb[:], float(THR), None,
                                            op0=ALU.is_ge)
                    nc.scalar.copy(g[:], zr[:, 0:W])
                    nc.vector.copy_predicated(g[:], msk[:], scsb[:])
                else:
                    # moments over the full [128, W] psum scores (in-place
                    # outs; the Square destroys sc after g is copied out)
                    s1 = smallp.tile([128, 1], F32, tag="s1")
                    s2 = smallp.tile([128, 1], F32, tag="s2")
                    nc.scalar.activation(sc[:], sc[:], AF.Copy, accum_out=s1[:])
                    nc.scalar.copy(g[:], sc[:])
                    nc.scalar.activation(sc[:], sc[:], AF.Square,
                                         accum_out=s2[:])
                    # t_est = max(mu + sd*z, 0.01)   (Pool engine, tiny ops)
                    mu = smallp.tile([128, 1], F32, tag="mu")
                    nc.vector.tensor_scalar(mu[:], s1[:], 1.0 / W, None,
                                            op0=ALU.mult)
                    mu2 = smallp.tile([128, 1], F32, tag="mu2")
                    nc.vector.tensor_tensor(mu2[:], mu[:], mu[:], op=ALU.mult)
                    var = smallp.tile([128, 1], F32, tag="varr")
                    nc.vector.tensor_scalar(var[:], s2[:], 1.0 / W, mu2[:, 0:1],
                                            op0=ALU.mult, op1=ALU.subtract)
                    sd = smallp.tile([128, 1], F32, tag="sd")
                    nc.scalar.activation(sd[:], var[:], AF.Sqrt)
                    tst = smallp.tile([128, 1], F32, tag="tst")
                    nc.vector.tensor_tensor(tst[:], sd[:], zqtt[:, c:c + 1],
                                            op=ALU.mult)
                    nc.vector.tensor_tensor(tst[:], tst[:], mu[:], op=ALU.add)
                    nc.vector.tensor_scalar(tst[:], tst[:], 0.01, None,
                                            op0=ALU.max)

                # causal NEG fill on the diagonal block
                nc.gpsimd.affine_select(
                    out=g[:, 128 * c:W], in_=g[:, 128 * c:W],
                    compare_op=ALU.is_gt, fill=float(NEG),
                    base=0, pattern=[[-1, 128]], channel_multiplier=1)

                if c >= 2:
                    # est-compaction: mask, prefix count, clamped scatter slots
                    m = gpool.tile([128, W], F32, tag="m")
                    nc.vector.tensor_scalar(m[:], g[:], tst[:, 0:1], None,
                                            op0=ALU.is_ge)
                    cnt = gpool.tile([128, W], F32, tag="cnt")
                    nc.vector.tensor_tensor_scan(
                        cnt[:], m[:], m[:], 0.0,
                        op0=ALU.add, op1=ALU.bypass)
                    t1 = gpool.tile([128, W], F32, tag="t1")
                    nc.vector.scalar_tensor_tensor(
                        t1[:], cnt[:], float(CAP), m[:], op0=ALU.is_le,
                        op1=ALU.mult)
                    scat = m
                    nc.vector.scalar_tensor_tensor(
                        scat[:], cnt[:], 1.0, t1[:], op0=ALU.mult, op1=ALU.mult)
                    # pair indices (2s, 2s+1) for 2-byte scatter of f32 g
                    sidx = tkpool.tile([128, 2 * W], I16, tag="sidx")
                    sv = sidx[:].rearrange("p (w two) -> p w two", two=2)
                    nc.vector.tensor_scalar(sv[:, :, 0:1], scat[:], 2.0, -2.0,
                                            op0=ALU.mult, op1=ALU.add)
                    nc.vector.tensor_scalar(sv[:, :, 1:2], scat[:], 2.0, -1.0,
                                            op0=ALU.mult, op1=ALU.add)
                    gc = tkpool.tile([128, 2 * max(CAP, 8 * R)], I16, tag="gc")
                    nc.gpsimd.local_scatter(
                        gc[:, 0:2 * CAP], g[:].bitcast(I16), sidx[:],
                        channels=128, num_elems=2 * CAP, num_idxs=2 * W)
                    gw = gc[:].bitcast(F32)
                    RW = CAP
                else:
                    gwt = tkpool.tile([128, max(W, 8 * R)], F32, tag="gwt")
                    nc.vector.tensor_copy(gwt[:, 0:W], g[:])
                    gw = gwt[:]
                    RW = W

                # max8/match_replace rounds to depth 8R
                vals = tkpool.tile([128, 8 * R], F32, tag="vals")
                for r in range(R):
                    sl = slice(8 * r, 8 * r + 8)
                    nc.vector.max(vals[:, sl], gw[:, 0:RW])
                    if r + 1 < R:
                        nc.vector.match_replace(gw[:, 0:RW], vals[:, sl],
                                                gw[:, 0:RW], float(NEG))

                # T = vals[k_i - 1] via fused one-hot dot
                tv = smallp.tile([128, OHW], F32, tag="tv")
                tthr = smallp.tile([128, 1], F32, tag="tthr")
                nc.vector.tensor_tensor(
                    tv[:, 0:8 * R], vals[:],
                    ohmt[:, c * OHW:c * OHW + 8 * R], op=ALU.mult)
                nc.vector.tensor_reduce(tthr[:], tv[:, 0:8 * R], axis=AX,
                                        op=ALU.add)
                if DEBUG:
                    nc.sync.dma_start(dbg["d_t"][0:128, 2 * c + h:2 * c + h + 1],
                                      tthr[:])
                    if c >= 2:
                        nc.sync.dma_start(
                            dbg["d_test"][0:128, 2 * c + h:2 * c + h + 1],
                            tst[:])

                # adjacency, bf16 (transposed next iteration)
                adj = gpool.tile([128, W], BF16, tag="adj", bufs=4)
                nc.vector.tensor_scalar(adj[:], g[:], tthr[:, 0:1], None,
                                        op0=ALU.is_ge)
                adjsb[(c, h)] = adj

                # chunk-0: index lists for the exact gather path (k <= 13)
                if c == 0:
                    cnt0 = smallp.tile([128, 128], F32, tag="cnt0")
                    nc.vector.tensor_tensor_scan(
                        cnt0[:], adj[:], adj[:], 0.0,
                        op0=ALU.add, op1=ALU.bypass)
                    t10 = smallp.tile([128, 128], F32, tag="t10")
                    nc.vector.scalar_tensor_tensor(
                        t10[:], cnt0[:], float(KP0), adj[:], op0=ALU.is_le,
                        op1=ALU.mult)
                    scat0 = smallp.tile([128, 128], F32, tag="scat0")
                    nc.vector.scalar_tensor_tensor(
                        scat0[:], cnt0[:], 1.0, t10[:], op0=ALU.mult,
                        op1=ALU.mult)
                    s0i = smallp.tile([128, 128], I16, tag="s0i")
                    nc.vector.tensor_scalar(s0i[:], scat0[:], 1.0, -1.0,
                                            op0=ALU.mult, op1=ALU.add)
                    ilist = smallp.tile([128, KP0], I16, tag="ilist")
                    nc.gpsimd.local_scatter(ilist[:], iot, s0i[:],
                                            channels=128, num_elems=KP0,
                                            num_idxs=128)
                    sc_dram = dscr.tile([128, KP0], I16, tag=f"scr{h}")
                    nc.sync.dma_start(sc_dram[0:128, 0:KP0], ilist[:])
                    tvals[(h, "ilist")] = sc_dram

        def vblock():
            """v projection and derived tables (vT, E, v_all, e_all)."""
            for n in range(2):
                sl = slice(512 * n, 512 * (n + 1))
                vp = mpsum.tile([128, 512], F32, tag="ps1")
                for k in range(8):
                    nc.tensor.matmul(vp[:], lhsT=wvt[k][:], rhs=hst[k][:, sl],
                                     start=(k == 0), stop=(k == 7))
                nc.scalar.copy(vT[:, sl], vp[:])
            nc.vector.tensor_scalar(epsv[:], vT[:], epst[:, 0:1], None,
                                    op0=ALU.mult)
            nc.vector.memset(vTg0[:, 0:1], NEG)
            nc.scalar.copy(vTg0[:, 1:129], vT[:, 0:128])
            for jb in range(NCHUNK):
                nc.vector.tensor_reduce(mbpos[:, jb:jb + 1],
                                        vT[:, 128 * jb:128 * (jb + 1)],
                                        axis=AX, op=ALU.max)
            nc.vector.tensor_scalar(mbneg[:], mbpos[:], -BETA, None,
                                    op0=ALU.mult)
            mbc = pers.tile([128, NCHUNK], F32, tag="mbc")
            nc.vector.tensor_scalar(mbc[:], mbpos[:], float(ECLIP / BETA),
                                    None, op0=ALU.subtract)
            nc.vector.tensor_scalar(mbpos[:], mbpos[:], float(LNS / BETA),
                                    None, op0=ALU.subtract)
            for jb in range(NCHUNK):
                vcl = ropep.tile([128, 128], F32, tag="vcl")
                nc.vector.tensor_scalar(vcl[:], vT[:, 128 * jb:128 * (jb + 1)],
                                        mbc[:, jb:jb + 1], None, op0=ALU.max)
                nc.scalar.activation(Ebf[:, 128 * jb:128 * (jb + 1)], vcl[:],
                                     AF.Exp, bias=mbneg[:, jb:jb + 1],
                                     scale=BETA)
            for jb in range(NCHUNK):
                tp2 = mpsum.tile([128, 128], F32, tag="ps1")
                nc.tensor.transpose(tp2[:], vT[:, 128 * jb:128 * (jb + 1)],
                                    identf[:])
                for h in range(2):
                    nc.scalar.copy(v_all[jb][:, 128 * h:128 * h + 64],
                                   tp2[:, 64 * h:64 * h + 64])
                    nc.scalar.activation(
                        v_all[jb][:, 128 * h + 64:128 * h + 128],
                        tp2[:, 64 * h:64 * h + 64], AF.Square)
                tpe = mpsum.tile([128, 128], BF16, tag="ps1")
                nc.tensor.transpose(tpe[:], Ebf[:, 128 * jb:128 * (jb + 1)],
                                    identb[:])
                nc.vector.tensor_copy(e_all[jb][:], tpe[:])

        def gather0():
            """chunk-0 exact max via ap_gather of the top-k v columns."""
            irep = gatp.tile([128, 8 * KP0], I16, tag="irep")
            for h in range(2):
                srcl = tvals[(h, "ilist")][0:128, 0:KP0]
                srcl = srcl.rearrange("(b q) s -> q b s", q=16)
                for gq in range(4):
                    g0 = (4 * h + gq) * 16
                    nc.sync.dma_start(
                        irep[g0:g0 + 16, :].rearrange("q (b s) -> q b s", b=8),
                        srcl)
            for b in range(8):
                gat = gatp.tile([128, 16 * KP0], F32, tag="gat")
                nc.gpsimd.ap_gather(
                    gat[:], vTg0[:], irep[:, b * KP0:(b + 1) * KP0],
                    channels=128, num_elems=129, d=1, num_idxs=16 * KP0)
                nc.vector.tensor_reduce(
                    comb_mx[:, 16 * b:16 * b + 16],
                    gat[:].rearrange("p (s r) -> p r s", r=16),
                    axis=AX, op=ALU.max)
            nc.vector.memset(comb_mx[:, 0:1], 0.0)

        def transp(c):
            for h in range(2):
                adj = adjsb.pop((c, h))
                for jb in range(c + 1):
                    tp = mpsum.tile([128, 128], BF16, tag="ps1")
                    nc.tensor.transpose(tp[:], adj[:, 128 * jb:128 * (jb + 1)],
                                        identb[:])
                    nc.vector.tensor_copy(
                        adjT[h][jb][:, 128 * (c - jb):128 * (c - jb) + 128],
                        tp[:])

        def phasec_mm(c):
            """aggregation matmuls + moments + LSE max for chunk c."""
            cc = slice(128 * c, 128 * (c + 1))
            for h in range(2):
                po = 64 * h
                pa = mpsum.tile([128, 128], F32, tag="ps1")
                for jb in range(c + 1):
                    lhs = v_all[jb][:, 128 * h:128 * (h + 1)]
                    nc.tensor.matmul(
                        pa[:], lhsT=lhs,
                        rhs=adjT[h][jb][:, 128 * (c - jb):128 * (c - jb) + 128],
                        start=(jb == 0), stop=(jb == c))
                nc.scalar.copy(comb_sum[po:po + 64, cc], pa[0:64, :])
                nc.vector.tensor_tensor(comb_mean[po:po + 64, cc], pa[0:64, :],
                                        rd[po:po + 64, cc], op=ALU.mult)
                varm = tmpp.tile([128, 128], F32, tag="varm")
                nc.vector.tensor_tensor(varm[po:po + 64, :], pa[64:128, :],
                                        rd[po:po + 64, cc], op=ALU.mult)
                msq = tmpp.tile([128, 128], F32, tag="msq")
                nc.scalar.activation(msq[po:po + 64, :],
                                     comb_mean[po:po + 64, cc], AF.Square)
                nc.vector.tensor_tensor(varm[po:po + 64, :],
                                        varm[po:po + 64, :],
                                        msq[po:po + 64, :], op=ALU.subtract)
                nc.vector.tensor_scalar(comb_var[po:po + 64, cc],
                                        varm[po:po + 64, :], 0.0, None,
                                        op0=ALU.max)

            # LSE max aggregator (chunks >= 1); sB oriented [d-part, i-free]
            if c >= 1:
                mxa = tmpp.tile([128, 128], BF16, tag="mxa")
                for jb in range(c + 1):
                    sB = mpsum.tile([128, 128], F32, tag="ps1")
                    for h in range(2):
                        nc.tensor.matmul(
                            sB[64 * h:64 * h + 64, :],
                            lhsT=e_all[jb][:, 64 * h:64 * h + 64],
                            rhs=adjT[h][jb][:,
                                            128 * (c - jb):128 * (c - jb) + 128],
                            start=True, stop=True)
                    lg = tmpp.tile([128, 128], F32, tag="lg")
                    nc.scalar.activation(lg[:], sB[:], AF.Ln,
                                         scale=float(np.exp(LNS)))
                    if jb == 0:
                        nc.vector.tensor_scalar(
                            mxa[:], lg[:], 1.0 / BETA, mbpos[:, 0:1],
                            op0=ALU.mult, op1=ALU.add)
                    else:
                        mxb = tmpp.tile([128, 128], BF16, tag="mxb")
                        nc.vector.tensor_scalar(
                            mxb[:], lg[:], 1.0 / BETA, mbpos[:, jb:jb + 1],
                            op0=ALU.mult, op1=ALU.add)
                        nc.vector.tensor_tensor(mxa[:], mxa[:], mxb[:],
                                                op=ALU.max)
                nc.vector.tensor_scalar(comb_mx[:, cc], mxa[:],
                                        float(MXGUARD), None, op0=ALU.max)

        # ---------------- phase D/E definitions (emitted in-loop) --------
        wpool = ctx.enter_context(tc.tile_pool(name="wmlp", bufs=1))
        h1pool = ctx.enter_context(tc.tile_pool(name="h1p", bufs=2))
        opool = ctx.enter_context(tc.tile_pool(name="op", bufs=1))
        w1t = {}
        w2t = {}
        for h in range(2):
            po = 64 * h
            w1t[h] = [wpool.tile([128, 128], BF16, tag=f"w1_{h}_{x}",
                                 name=f"w1t{h}{x}") for x in range(4)]
            for x in range(4):
                dma(w1t[h][x][po:po + 64, :], w1b[h, 64 * x:64 * (x + 1), :])
            w2t[h] = wpool.tile([128, 64], BF16, tag=f"w2_{h}", name=f"w2t{h}")
            dma(w2t[h][:], w2b[h])
        wot = pers.tile([128, S], BF16, tag="wot")
        for n in range(2):
            dma(wot[:, 512 * n:512 * (n + 1)], wob[:, 512 * n:512 * (n + 1)])

        def mlp_oproj(c):
            sl = slice(128 * c, 128 * (c + 1))
            combs = [comb_sum, comb_mean, comb_mx, comb_var]
            for h in range(2):
                po = 64 * h
                h1p = mpsum.tile([128, 128], F32, tag="ps1")
                for x in range(4):
                    nc.tensor.matmul(h1p[:], lhsT=w1t[h][x][po:po + 64, :],
                                     rhs=combs[x][po:po + 64, sl],
                                     start=(x == 0), stop=(x == 3))
                h1sb = h1pool.tile([128, 128], BF16, tag="h1sb")
                nc.scalar.activation(h1sb[:], h1p[:], AF.Silu)
                hop = mpsum.tile([64, 128], F32, tag="ps1")
                nc.tensor.matmul(hop[:], lhsT=w2t[h][:], rhs=h1sb[:],
                                 start=True, stop=True)
                nc.vector.tensor_tensor(houtT[po:po + 64, sl], hop[:],
                                        epsv[po:po + 64, sl], op=ALU.add)
            osb = opool.tile([128, S], BF16, tag="osb")
            for n in range(2):
                nsl = slice(512 * n, 512 * (n + 1))
                op = mpsum.tile([128, 512], F32, tag="ps1")
                nc.tensor.matmul(op[:], lhsT=houtT[:, sl],
                                 rhs=wot[:, nsl], start=True, stop=True)
                if n == 0:
                    nc.scalar.copy(osb[:, nsl], op[:])
                else:
                    nc.vector.tensor_copy(osb[:, nsl], op[:])
            for n in range(2):
                dma(outp[128 * c:128 * (c + 1), 512 * n:512 * (n + 1)],
                    osb[:, 512 * n:512 * (n + 1)])

        # ---- software-pipelined emission ----
        proj_rope(0, wkt, kTr, tk, tsk_t)
        proj_rope(0, wqt, qTr, tq, tsq_t)
        for c in range(NCHUNK):
            if c + 1 < NCHUNK:
                proj_rope(c + 1, wkt, kTr, tk, tsk_t)
                proj_rope(c + 1, wqt, qTr, tq, tsq_t)
            sel_chunk(c)
            if c == 1:
                vblock()
                transp(0)
                phasec_mm(0)
                gather0()
                mlp_oproj(0)
            elif c >= 2:
                transp(c - 1)
                phasec_mm(c - 1)
                mlp_oproj(c - 1)
        transp(7)
        phasec_mm(7)
        mlp_oproj(7)

        # ---------------- phase C: aggregation + moments + LSE max ----------
        tmpp = ctx.enter_context(tc.tile_pool(name="tmpp", bufs=2))
        for c in range(NCHUNK):
            cc = slice(128 * c, 128 * (c + 1))
            for h in range(2):
                po = 64 * h
                pa = mpsum.tile([128, 128], F32, tag="ps1")
                for jb in range(c + 1):
                    lhs = v_all[jb][:, 128 * h:128 * (h + 1)]
                    nc.tensor.matmul(
                        pa[:], lhsT=lhs,
                        rhs=adjT[h][jb][:, 128 * (c - jb):128 * (c - jb) + 128],
                        start=(jb == 0), stop=(jb == c))
                nc.scalar.copy(comb_sum[po:po + 64, cc], pa[0:64, :])
                nc.vector.tensor_tensor(comb_mean[po:po + 64, cc], pa[0:64, :],
                                        rd[po:po + 64, cc], op=ALU.mult)
                varm = tmpp.tile([128, 128], F32, tag="varm")
                nc.vector.tensor_tensor(varm[po:po + 64, :], pa[64:128, :],
                                        rd[po:po + 64, cc], op=ALU.mult)
                msq = tmpp.tile([128, 128], F32, tag="msq")
                nc.scalar.activation(msq[po:po + 64, :],
                                     comb_mean[po:po + 64, cc], AF.Square)
                nc.vector.tensor_tensor(varm[po:po + 64, :], varm[po:po + 64, :],
                                        msq[po:po + 64, :], op=ALU.subtract)
                nc.vector.tensor_scalar(comb_var[po:po + 64, cc],
                                        varm[po:po + 64, :], 0.0, None,
                                        op0=ALU.max)

            # LSE max aggregator (chunks >= 1); sB oriented [d-part, i-free]
            # so the per-block center is a per-partition scalar.
            if c >= 1:
                mxa = tmpp.tile([128, 128], BF16, tag="mxa")
                for jb in range(c + 1):
                    sB = mpsum.tile([128, 128], F32, tag="ps1")
                    for h in range(2):
                        nc.tensor.matmul(
                            sB[64 * h:64 * h + 64, :],
                            lhsT=e_all[jb][:, 64 * h:64 * h + 64],
                            rhs=adjT[h][jb][:,
                                            128 * (c - jb):128 * (c - jb) + 128],
                            start=True, stop=True)
                    lg = tmpp.tile([128, 128], F32, tag="lg")
                    nc.scalar.activation(lg[:], sB[:], AF.Ln,
                                         scale=float(np.exp(LNS)))
                    if jb == 0:
                        nc.vector.tensor_scalar(
                            mxa[:], lg[:], 1.0 / BETA, mbpos[:, 0:1],
                            op0=ALU.mult, op1=ALU.add)
                    else:
                        mxb = tmpp.tile([128, 128], BF16, tag="mxb")
                        nc.vector.tensor_scalar(
                            mxb[:], lg[:], 1.0 / BETA, mbpos[:, jb:jb + 1],
                            op0=ALU.mult, op1=ALU.add)
                        nc.vector.tensor_tensor(mxa[:], mxa[:], mxb[:],
                                                op=ALU.max)
                nc.vector.tensor_scalar(comb_mx[:, cc], mxa[:],
                                        float(MXGUARD), None, op0=ALU.max)


        if DEBUG:
            for nm, t in (("d_sum", comb_sum), ("d_mean", comb_mean),
                          ("d_mx", comb_mx), ("d_var", comb_var),
                          ("d_hout", houtT)):
                tf = gpool.tile([128, S], F32, tag="g")
                nc.vector.tensor_copy(tf[:], t[:])
                nc.sync.dma_start(dbg[nm], tf[:])

    nc.compile()
    return nc


def _norm_ppf(p):
    """Acklam's inverse normal CDF approximation (|err| < 1.2e-9)."""
    p = np.asarray(p, dtype=np.float64)
    a = [-3.969683028665376e+01, 2.209460984245205e+02, -2.759285104469687e+02,
         1.383577518672690e+02, -3.066479806614716e+01, 2.506628277459239e+00]
    b = [-5.447609879822406e+01, 1.615858368580409e+02, -1.556989798598866e+02,
         6.680131188771972e+01, -1.328068155288572e+01]
    c = [-7.784894002430293e-03, -3.223964580411365e-01, -2.400758277161838e+00,
         -2.549732539343734e+00, 4.374664141464968e+00, 2.938163982698783e+00]
    d = [7.784695709041462e-03, 3.224671290700398e-01, 2.445134137142996e+00,
         3.754408661907416e+00]
    plow, phigh = 0.02425, 1 - 0.02425
    out = np.empty_like(p)
    lo = p < plow
    hi = p > phigh
    mid = ~(lo | hi)
    if lo.any():
        q = np.sqrt(-2 * np.log(p[lo]))
        out[lo] = ((((((c[0] * q + c[1]) * q + c[2]) * q + c[3]) * q + c[4]) * q
                    + c[5]) /
                   ((((d[0] * q + d[1]) * q + d[2]) * q + d[3]) * q + 1))
    if hi.any():
        q = np.sqrt(-2 * np.log(1 - p[hi]))
        out[hi] = -((((((c[0] * q + c[1]) * q + c[2]) * q + c[3]) * q + c[4]) * q
                     + c[5]) /
                    ((((d[0] * q + d[1]) * q + d[2]) * q + d[3]) * q + 1))
    if mid.any():
        q = p[mid] - 0.5
        r = q * q
        out[mid] = ((((((a[0] * r + a[1]) * r + a[2]) * r + a[3]) * r + a[4]) * r
                     + a[5]) * q /
                    (((((b[0] * r + b[1]) * r + b[2]) * r + b[3]) * r + b[4]) * r
                     + 1))
    return out


def _host_inputs(inputs):
    """Build the 8 per-core input dicts from the full problem inputs."""
    hs = np.ascontiguousarray(np.asarray(inputs["hidden_states"],
                                         dtype=np.float32)[0])      # (S, HID)
    Wq = np.asarray(inputs["Wq"], dtype=np.float32)
    Wk = np.asarray(inputs["Wk"], dtype=np.float32)
    Wv = np.asarray(inputs["Wv"], dtype=np.float32)
    Wo = np.asarray(inputs["Wo"], dtype=np.float32)
    W1 = np.asarray(inputs["W1"], dtype=np.float32)
    W2 = np.asarray(inputs["W2"], dtype=np.float32)
    eps = np.float32(np.asarray(inputs["eps"]).reshape(-1)[0])
    pos = np.asarray(inputs["position_ids"]).reshape(-1).astype(np.float32)

    import ml_dtypes
    bf = lambda a: np.ascontiguousarray(a).astype(ml_dtypes.bfloat16)

    hsT = np.ascontiguousarray(hs.T)

    inv = (1.0 / (np.float32(BASE) **
                  (np.arange(0, D, 2, dtype=np.float32) / np.float32(D))))
    ang = pos[:, None] * inv[None, :].astype(np.float32)            # (S, 32)
    c32 = np.cos(ang).astype(np.float32).T                          # (32, S)
    s32 = np.sin(ang).astype(np.float32).T
    stack = lambda a: np.concatenate([a, a, a, a], axis=0)          # (128, S)
    tcq = stack((c32 / np.float32(8.0)).astype(np.float32))
    tsq = stack((s32 / np.float32(8.0)).astype(np.float32))
    tck = stack(c32)
    tsk = stack(s32)

    j = np.arange(S, dtype=np.float32)
    zrow = (np.float32(DELTA) * (np.float32(S) - j)).astype(np.float32)
    zrep = np.broadcast_to(zrow[:256], (128, 256)).copy()

    denom = np.maximum(KV, 1).astype(np.float32)
    rden = np.broadcast_to((np.float32(1.0) / denom), (128, S)).copy()

    epsc = np.full((128, 1), eps, dtype=np.float32)
    ropes = np.concatenate([tck, tsk, tcq, tsq], axis=1)

    pmat = np.zeros((128, 128), dtype=np.float32)
    for h in range(2):
        b = 64 * h
        for r in range(32):
            pmat[b + 32 + r, b + r] = -1.0
            pmat[b + r, b + 32 + r] = 1.0

    # one-hot at col k_i-1 (k_i=0 -> all-zero row), packed [128, c*OHW+w]
    ohm = np.zeros((128, NCHUNK * OHW), dtype=np.float32)
    for c in range(NCHUNK):
        for r in range(128):
            k = int(KV[128 * c + r])
            if k > 0:
                ohm[r, c * OHW + k - 1] = 1.0

    # Gaussian z per row for target count = (k_i + CAP)/2 among i candidates
    zqt = np.zeros((128, NCHUNK), dtype=np.float32)
    for c in range(2, NCHUNK):
        i_idx = np.arange(128 * c, 128 * (c + 1)).astype(np.float64)
        target = (KV[128 * c:128 * (c + 1)].astype(np.float64) + CAPS[c]) / 2.0
        zqt[:, c] = _norm_ppf(1.0 - target / i_idx).astype(np.float32)

    iotp1 = np.broadcast_to((np.arange(128) + 1).astype(np.int16),
                            (128, 128)).copy()
    blob = np.zeros((128, BLOBW), dtype=np.float32)
    blob[:, BO_ZR:BO_ZR + 256] = zrep
    blob[:, BO_RD:BO_RD + 1024] = rden
    blob[:, BO_OHM:BO_OHM + 896] = ohm
    blob[:, BO_ZQ:BO_ZQ + 8] = zqt
    blob[:, BO_EPS] = eps
    blob[:, BO_PM:BO_PM + 128] = pmat
    blob[:, BO_IOT:BO_IOT + 64] = iotp1.view(np.float32)

    maps = []
    for core in range(NCORES):
        h0 = 2 * core
        sl = slice(h0 * D, (h0 + 2) * D)
        maps.append({
            "hsT": hsT,
            "wq": np.ascontiguousarray(Wq[:, sl]),
            "wk": np.ascontiguousarray(Wk[:, sl]),
            "wv": np.ascontiguousarray(Wv[:, sl]),
            "wob": bf(Wo[sl, :]),
            "w1b": bf(W1[h0:h0 + 2]),
            "w2b": bf(W2[h0:h0 + 2]),
            "ropes": ropes, "blob": blob,
        })
    return maps


_NC_CACHE = {}


def _get_nc():
    if "nc" not in _NC_CACHE:
        _NC_CACHE["nc"] = _build_nc()
    return _NC_CACHE["nc"]


def _get_runner():
    """Compile once; return (fn, in_names, zero_outs, mesh/sharding)."""
    if "runner" in _NC_CACHE:
        return _NC_CACHE["runner"]
    import jax
    from jax.sharding import Mesh, PartitionSpec, NamedSharding
    from jax.experimental.shard_map import shard_map
    from concourse import bass2jax

    nc = _get_nc()
    bass2jax.install_neuronx_cc_hook()
    partition_name = (nc.partition_id_tensor.name
                      if nc.partition_id_tensor else None)
    in_names, out_names, out_avals, zero_outs = [], [], [], []
    for alloc in nc.m.functions[0].allocations:
        if not isinstance(alloc, mybir.MemoryLocationSet):
            continue
        name = alloc.memorylocations[0].name
        if alloc.kind == "ExternalInput":
            if name != partition_name:
                in_names.append(name)
        elif alloc.kind == "ExternalOutput":
            out_names.append(name)
            shape = tuple(alloc.tensor_shape)
            dtype = mybir.dt.np(alloc.dtype)
            out_avals.append(jax.core.ShapedArray(shape, dtype))
            zero_outs.append(np.zeros(shape, dtype))
    all_in = in_names + out_names + ([partition_name] if partition_name else [])

    def _body(*args):
        ops = list(args)
        if partition_name:
            ops.append(bass2jax.partition_id_tensor())
        return tuple(bass2jax._bass_exec_p.bind(
            *ops, out_avals=tuple(out_avals), in_names=tuple(all_in),
            out_names=tuple(out_names), lowering_input_output_aliases=(),
            sim_require_finite=True, sim_require_nnan=True, nc=nc))

    devices = jax.devices()[:NCORES]
    mesh = Mesh(np.asarray(devices), ("core",))
    spec = PartitionSpec("core")
    fn = jax.jit(shard_map(
        _body, mesh=mesh,
        in_specs=(spec,) * (len(in_names) + len(out_names)),
        out_specs=(spec,) * len(out_names), check_rep=False))
    sh = NamedSharding(mesh, spec)
    zo_dev = [jax.device_put(np.concatenate([zo] * NCORES, axis=0), sh)
              for zo in zero_outs]
    _NC_CACHE["runner"] = (fn, in_names, zo_dev, sh, jax)
    return _NC_CACHE["runner"]


def kernel(**inputs) -> np.ndarray:
    fn, in_names, zo_dev, sh, jax = _get_runner()
    maps = _host_inputs(inputs)
    args = []
    for name in in_names:
        ci = np.concatenate([np.asarray(maps[c][name]) for c in range(NCORES)],
                            axis=0)
        args.append(jax.device_put(ci, sh))
    args.extend(zo_dev)
    outs = fn(*args)
    import jax.numpy as jnp
    full = np.asarray(jnp.asarray(outs[0], dtype=jnp.float32))
    out = full.reshape(NCORES, S, S).sum(axis=0, dtype=np.float32)
    return out[None].astype(np.float32)
